# revision 7
# baseline (speedup 1.0000x reference)
"""PiGNNLayer Trainium2 Bass kernel.

Computes the reference nn_PiGNNLayer graph-attention layer on 8 NeuronCores.

Sharding: core c owns nodes [c*N/8, (c+1)*N/8) and their contiguous K=30-edge
blocks.  All MLPs / softmax / weighted sums are local to a node's edge block;
the scatter-mean for the gate needs one 512-byte AllReduce.

On-chip layout is feature-major: activations live as [128 features, edges]
tiles so every Linear layer is a weights-stationary matmul
(out_T = W.T @ x_T via matmul(out, lhsT=W, rhs=x_T)).  Edge features e are
pre-transposed on the host; h_dst rows are gathered on-device with
dma_gather(transpose=True) (bf16, 16-bit-granular transpose) straight into
feature-major tiles.  h_src is the node's own row repeated K times, so its
first-layer contribution (h @ aw1[:128] + ab1) is precomputed per node on the
host and added into the layer-1 PSUM with an identity matmul whose rhs uses a
stride-0 column-repeat access pattern.

Softmax over the K=30 neighbors skips the max-subtraction (logits are O(1) by
construction), exponentiates attention logits that were replicated across all
128 partitions by tiling the aw3 column 128x, and defers normalization to
after the attention-weighted K-sum.  node_mlp layer 3 and to_h are fused into
a single node-level matmul with W' = nw3 @ thw (host-precomputed) because the
einsum commutes with the last linear layer (and sum(att)=1 handles the bias).
"""

import sys
import os

for _p in ("/opt/trn_rl_repo",):
    if _p not in sys.path and os.path.isdir(_p):
        sys.path.insert(0, _p)

import numpy as np
import ml_dtypes
from contextlib import ExitStack

import concourse.bass as bass
import concourse.bacc as bacc
import concourse.tile as tile
import concourse.mybir as mybir
from concourse.bass_utils import run_bass_kernel_spmd
from concourse.library_config import mlp as _mlp_lib

AF = mybir.ActivationFunctionType
F32 = mybir.dt.float32
F32R = mybir.dt.float32r
BF16 = mybir.dt.bfloat16
I16 = mybir.dt.int16

# Problem shape (hardcoded per spec).
N, K, D, H = 16384, 30, 128, 1
NCORES = 8
NLOC = N // NCORES            # nodes per core
ELOC = NLOC * K               # edges per core
BLKN = 16                     # nodes per matmul block
BLK = BLKN * K                # 480 free-dim columns per matmul block
SCN = 64                      # nodes per super-chunk
SCC = SCN * K                 # 1920 columns per super-chunk
NBLK = SCC // BLK             # 4 blocks per super-chunk
NSC = NLOC // SCN             # super-chunks per core
BATCH = 4                     # super-chunks per att/node phase batch
NBATCH = NSC // BATCH
SCALE = 1.0 / float(np.sqrt(D // H))

_CACHE = {}


def _build(ncores=NCORES, nloc=NLOC, use_collective=True, use_gather=True):
    nsc = nloc // SCN
    nbatch = nsc // BATCH
    eloc = nloc * K
    nc = bacc.Bacc("TRN2", target_bir_lowering=False, debug=False,
                   num_devices=ncores)

    def din(name, shape, dt):
        return nc.dram_tensor(name, shape, dt, kind="ExternalInput").ap()

    eT = din("eT", [128, eloc], F32R)
    hb = din("hb", [N if nloc == NLOC else nloc * ncores, 128], BF16)
    idx = din("idx", [128, eloc // 16], I16)
    ha1 = din("ha1", [128, nloc], F32R)
    # weights
    aw1e = din("aw1e", [128, 128], F32R)
    aw1h = din("aw1h", [128, 128], BF16)
    ident = din("identw", [128, 128], F32R)
    identf = din("identf", [128, 128], F32)
    aw2 = din("aw2", [128, 128], F32R)
    aw3r = din("aw3r", [128, 128], F32R)
    nw1e = din("nw1e", [128, 128], F32R)
    nw1h = din("nw1h", [128, 128], BF16)
    nw2 = din("nw2", [128, 128], F32R)
    wp = din("wp", [128, 128], F32R)
    gw1 = din("gw1", [128, 128], F32)
    gw2 = din("gw2", [128, 128], F32)
    gw3 = din("gw3", [128, 128], F32)
    # bias vectors packed [128, 7]: ab2, nb1, nb2, bp, gb1, gb2, gb3/2
    bvec = din("bvec", [128, 7], F32)
    out = nc.dram_tensor("out", [nloc, 128], F32, kind="ExternalOutput").ap()

    with tile.TileContext(nc) as tc, ExitStack() as ctx:
        wpool = ctx.enter_context(tc.tile_pool(name="wpool", bufs=1))
        per = ctx.enter_context(tc.tile_pool(name="per", bufs=1))
        epool = ctx.enter_context(tc.tile_pool(name="epool", bufs=BATCH + 1))
        jpool = ctx.enter_context(tc.tile_pool(name="jpool", bufs=BATCH + 1))
        ppool = ctx.enter_context(tc.tile_pool(name="ppool", bufs=BATCH))
        xpool = ctx.enter_context(tc.tile_pool(name="xpool", bufs=2))
        y2pool = ctx.enter_context(tc.tile_pool(name="y2pool", bufs=2))
        opool = ctx.enter_context(tc.tile_pool(name="opool", bufs=2))
        psA = ctx.enter_context(tc.tile_pool(name="psA", bufs=NBLK, space="PSUM"))
        psB = ctx.enter_context(tc.tile_pool(name="psB", bufs=2, space="PSUM"))
        psC = ctx.enter_context(tc.tile_pool(name="psC", bufs=2, space="PSUM"))
        dram = ctx.enter_context(tc.tile_pool(name="dram", bufs=1, space="DRAM"))

        # --- load weights / persistent data -------------------------------
        def wtile(src, dt):
            t = wpool.tile([128, src.shape[1]], dt, name=f"w_{src.tensor.name}")
            nc.sync.dma_start(t[:], src[:])
            return t

        w_aw1e = wtile(aw1e, F32R)
        w_aw1h = wtile(aw1h, BF16)
        w_ident = wtile(ident, F32R)
        w_identf = wtile(identf, F32)
        w_aw2 = wtile(aw2, F32R)
        w_aw3r = wtile(aw3r, F32R)
        w_nw1e = wtile(nw1e, F32R)
        w_nw1h = wtile(nw1h, BF16)
        w_nw2 = wtile(nw2, F32R)
        w_wp = wtile(wp, F32R)
        w_gw1 = wtile(gw1, F32)
        w_gw2 = wtile(gw2, F32)
        w_gw3 = wtile(gw3, F32)
        w_bias = wtile(bvec, F32)
        b_ab2 = w_bias[:, 0:1]
        b_nb1 = w_bias[:, 1:2]
        b_nb2 = w_bias[:, 2:3]
        b_bp = w_bias[:, 3:4]
        b_gb1 = w_bias[:, 4:5]
        b_gb2 = w_bias[:, 5:6]
        b_gb3h = w_bias[:, 6:7]

        ha1_sb = per.tile([128, nloc], F32R)
        nc.sync.dma_start(ha1_sb[:], ha1[:])
        idx_sb = per.tile([128, eloc // 16], I16)
        nc.sync.dma_start(idx_sb[:], idx[:])
        u_sb = per.tile([128, nloc], F32)
        den_sb = per.tile([128, nloc], F32)
        hh_sb = per.tile([128, nloc], F32)

        nc.gpsimd.load_library(_mlp_lib)

        # --- main loop ----------------------------------------------------
        for bi in range(nbatch):
            e_tiles, j_tiles, p_tiles = [], [], []
            # fetch inputs for this batch of super-chunks
            for sj in range(BATCH):
                sc = bi * BATCH + sj
                c0 = sc * SCC
                et = epool.tile([128, SCC], F32R, name=f"et{bi}_{sj}", tag="et")
                nc.sync.dma_start(et[:], eT[:, c0:c0 + SCC])
                jt = jpool.tile([128, SCC], BF16, name=f"jt{bi}_{sj}", tag="jt")
                if use_gather:
                    # transpose-mode dma_gather is limited to <=768 idxs/call
                    GC = 640
                    for gj in range(SCC // GC):
                        i0 = sc * (SCC // 16) + gj * (GC // 16)
                        nc.gpsimd.dma_gather(
                            jt[:, gj * GC:(gj + 1) * GC].unsqueeze(1), hb[:],
                            idx_sb[:, i0:i0 + GC // 16],
                            GC, GC, 128, transpose=True)
                else:
                    nc.gpsimd.memset(jt[:], 1.0)
                e_tiles.append(et)
                j_tiles.append(jt)

            # ---- attention phase over the batch --------------------------
            for sj in range(BATCH):
                sc = bi * BATCH + sj
                n0 = sc * SCN
                et, jt = e_tiles[sj], j_tiles[sj]
                pt = ppool.tile([128, SCC], BF16, name=f"pt{bi}_{sj}", tag="pt")
                p_tiles.append(pt)

                a1 = [psA.tile([128, BLK], F32, name=f"a1_{sc}_{b}", tag="A")
                      for b in range(NBLK)]
                for b in range(NBLK):
                    nc.tensor.matmul(a1[b][:], w_aw1e[:], et[:, b * BLK:(b + 1) * BLK],
                                     start=True, stop=False)
                for b in range(NBLK):
                    nc.tensor.matmul(a1[b][:], w_aw1h[:], jt[:, b * BLK:(b + 1) * BLK],
                                     start=False, stop=False)
                for b in range(NBLK):
                    rep = ha1_sb[:, n0 + b * BLKN:n0 + (b + 1) * BLKN] \
                        .unsqueeze(2).broadcast_to([128, BLKN, K])
                    nc.tensor.matmul(
                        a1[b][:].rearrange("p (n k) -> p n k", n=BLKN),
                        w_ident[:], rep, start=False, stop=True)

                x1 = xpool.tile([128, SCC], F32R, name=f"x1_{sc}", tag="x")
                for b in range(NBLK):
                    sl = slice(b * BLK, (b + 1) * BLK)
                    if sc % 2 == 0:
                        nc.scalar.activation(x1[:, sl], a1[b][:], AF.Relu)
                    else:
                        nc.vector.tensor_scalar_max(x1[:, sl], a1[b][:], 0.0)

                x2 = xpool.tile([128, SCC], F32R, name=f"x2_{sc}", tag="x")
                for b in range(NBLK):
                    sl = slice(b * BLK, (b + 1) * BLK)
                    a2 = psB.tile([128, BLK], F32, name=f"a2_{sc}_{b}", tag="B")
                    nc.tensor.matmul(a2[:], w_aw2[:], x1[:, sl], start=True, stop=True)
                    nc.vector.tensor_scalar(x2[:, sl], a2[:], b_ab2, 0.0,
                                            mybir.AluOpType.add, mybir.AluOpType.max)

                for b in range(NBLK):
                    sl = slice(b * BLK, (b + 1) * BLK)
                    a3 = psC.tile([128, BLK], F32, name=f"a3_{sc}_{b}", tag="C")
                    nc.tensor.matmul(a3[:], w_aw3r[:], x2[:, sl], start=True, stop=True)
                    nc.scalar.activation(pt[:, sl], a3[:], AF.Exp, scale=SCALE)

                nc.vector.reduce_sum(
                    den_sb[:, n0:n0 + SCN],
                    pt[:].rearrange("p (n k) -> p n k", n=SCN),
                    axis=mybir.AxisListType.X)

            # ---- node-value phase over the batch -------------------------
            for sj in range(BATCH):
                sc = bi * BATCH + sj
                n0 = sc * SCN
                et, jt, pt = e_tiles[sj], j_tiles[sj], p_tiles[sj]

                n1 = [psA.tile([128, BLK], F32, name=f"n1_{sc}_{b}", tag="A")
                      for b in range(NBLK)]
                for b in range(NBLK):
                    nc.tensor.matmul(n1[b][:], w_nw1e[:], et[:, b * BLK:(b + 1) * BLK],
                                     start=True, stop=False)
                for b in range(NBLK):
                    nc.tensor.matmul(n1[b][:], w_nw1h[:], jt[:, b * BLK:(b + 1) * BLK],
                                     start=False, stop=True)

                y1 = xpool.tile([128, SCC], F32R, name=f"y1_{sc}", tag="x")
                for b in range(NBLK):
                    sl = slice(b * BLK, (b + 1) * BLK)
                    nc.scalar.activation(y1[:, sl], n1[b][:], AF.Gelu, bias=b_nb1)

                y2 = y2pool.tile([128, SCC], BF16, name=f"y2_{sc}", tag="y2")
                for b in range(NBLK):
                    sl = slice(b * BLK, (b + 1) * BLK)
                    n2 = psB.tile([128, BLK], F32, name=f"n2_{sc}_{b}", tag="B")
                    nc.tensor.matmul(n2[:], w_nw2[:], y1[:, sl], start=True, stop=True)
                    nc.scalar.activation(y2[:, sl], n2[:], AF.Gelu, bias=b_nb2)

                # p <- p * y2, then K-group sum into u
                nc.vector.tensor_tensor(pt[:], y2[:], pt[:], mybir.AluOpType.mult)
                nc.vector.reduce_sum(
                    u_sb[:, n0:n0 + SCN],
                    pt[:].rearrange("p (n k) -> p n k", n=SCN),
                    axis=mybir.AxisListType.X)

        # --- tail: normalize, project, gate, transpose, store -------------
        deni = per.tile([128, nloc], F32)
        nc.vector.reciprocal(deni[:], den_sb[:])
        ubar = per.tile([128, nloc], F32R)
        nc.vector.tensor_tensor(ubar[:], u_sb[:], deni[:], mybir.AluOpType.mult)

        for b in range(nloc // 512):
            sl = slice(b * 512, (b + 1) * 512)
            ph = psB.tile([128, 512], F32, name=f"hh_{b}", tag="B")
            nc.tensor.matmul(ph[:], w_wp[:], ubar[:, sl], start=True, stop=True)
            nc.scalar.activation(hh_sb[:, sl], ph[:], AF.Identity, bias=b_bp)

        csum = per.tile([128, 1], F32)
        nc.vector.reduce_sum(csum[:], hh_sb[:], axis=mybir.AxisListType.X)

        cin = dram.tile([128, 1], F32)
        cout = dram.tile([128, 1], F32,
                         addr_space="Shared" if ncores > 4 else "Local")
        nc.sync.dma_start(cin[:], csum[:])
        if use_collective:
            nc.gpsimd.collective_compute(
                "AllReduce", mybir.AluOpType.add,
                replica_groups=[list(range(ncores))],
                ins=[cin[:].opt()], outs=[cout[:].opt()])
        else:
            nc.sync.dma_start(cout[:], cin[:])
        call = per.tile([128, 1], F32)
        nc.sync.dma_start(call[:], cout[:])

        # gate MLP (redundant on every core), sigmoid via tanh
        g1p = psC.tile([128, 1], F32, name="g1p", tag="C")
        nc.tensor.matmul(g1p[:], w_gw1[:], call[:], start=True, stop=True)
        g1 = per.tile([128, 1], F32)
        nc.scalar.activation(g1[:], g1p[:], AF.Relu, bias=b_gb1,
                             scale=1.0 / float(ncores * nloc))
        g2p = psC.tile([128, 1], F32, name="g2p", tag="C")
        nc.tensor.matmul(g2p[:], w_gw2[:], g1[:], start=True, stop=True)
        g2 = per.tile([128, 1], F32)
        nc.scalar.activation(g2[:], g2p[:], AF.Relu, bias=b_gb2)
        g3p = psC.tile([128, 1], F32, name="g3p", tag="C")
        nc.tensor.matmul(g3p[:], w_gw3[:], g2[:], start=True, stop=True)
        gth = per.tile([128, 1], F32)
        nc.scalar.activation(gth[:], g3p[:], AF.Tanh, bias=b_gb3h, scale=0.5)
        gv = per.tile([128, 1], F32)
        nc.vector.tensor_scalar(gv[:], gth[:], 0.5, 0.5,
                                mybir.AluOpType.mult, mybir.AluOpType.add)

        nc.vector.tensor_scalar(hh_sb[:], hh_sb[:], gv[:], None,
                                mybir.AluOpType.mult)

        for b in range(nloc // 128):
            sl = slice(b * 128, (b + 1) * 128)
            pt_ps = psB.tile([128, 128], F32, name=f"tr_{b}", tag="B")
            nc.tensor.transpose(pt_ps[:], hh_sb[:, sl], w_identf[:])
            ot = opool.tile([128, 128], F32, name=f"ot_{b}", tag="ot")
            nc.vector.tensor_copy(ot[:], pt_ps[:])
            nc.sync.dma_start(out[sl, :], ot[:])

    nc.compile()
    return nc


def _prep_inputs(h, e, aw1, ab1, aw2, ab2, aw3, ab3,
                 nw1, nb1, nw2, nb2, nw3, nb3, thw,
                 gw1, gb1, gw2, gb2, gw3, gb3,
                 edge_idx, batch_idx, ncores=NCORES, nloc=NLOC):
    n = ncores * nloc
    eloc = nloc * K
    src = np.asarray(edge_idx[0])
    assert np.array_equal(src, np.repeat(np.arange(n, dtype=src.dtype), K)), \
        "kernel assumes edge_idx[0] == repeat(arange(N), K)"
    assert np.all(np.asarray(batch_idx) == 0), "kernel assumes batch_idx == 0"
    dst = np.asarray(edge_idx[1]).astype(np.int16)

    h = np.asarray(h, np.float32)
    e = np.asarray(e, np.float32)
    eT = np.ascontiguousarray(e.T)                      # [128, E]
    hb = np.ascontiguousarray(h.astype(ml_dtypes.bfloat16))
    ha1 = np.ascontiguousarray((h @ np.asarray(aw1)[:D] + np.asarray(ab1)).T)
    wp = np.asarray(nw3, np.float32) @ np.asarray(thw, np.float32)
    bp = np.asarray(nb3, np.float32) @ np.asarray(thw, np.float32)
    aw3r = np.ascontiguousarray(np.tile(np.asarray(aw3, np.float32), (1, 128)))
    identw = np.eye(128, dtype=np.float32)

    bvec = np.stack([
        np.asarray(ab2, np.float32), np.asarray(nb1, np.float32),
        np.asarray(nb2, np.float32), bp,
        np.asarray(gb1, np.float32), np.asarray(gb2, np.float32),
        np.asarray(gb3, np.float32) * 0.5,
    ], axis=1)                                          # [128, 7]

    common = {
        "hb": hb,
        "aw1e": np.ascontiguousarray(np.asarray(aw1, np.float32)[D:2 * D]),
        "aw1h": np.ascontiguousarray(
            np.asarray(aw1, np.float32)[2 * D:3 * D].astype(ml_dtypes.bfloat16)),
        "identw": identw, "identf": identw,
        "aw2": np.asarray(aw2, np.float32),
        "aw3r": aw3r,
        "nw1e": np.ascontiguousarray(np.asarray(nw1, np.float32)[:D]),
        "nw1h": np.ascontiguousarray(
            np.asarray(nw1, np.float32)[D:2 * D].astype(ml_dtypes.bfloat16)),
        "nw2": np.asarray(nw2, np.float32),
        "wp": wp,
        "gw1": np.asarray(gw1, np.float32),
        "gw2": np.asarray(gw2, np.float32),
        "gw3": np.asarray(gw3, np.float32),
        "bvec": np.ascontiguousarray(bvec),
    }

    in_maps = []
    for c in range(ncores):
        dc = dst[c * eloc:(c + 1) * eloc]
        iw = np.ascontiguousarray(np.tile(dc.reshape(-1, 16).T, (8, 1)))
        m = dict(common)
        m["eT"] = np.ascontiguousarray(eT[:, c * eloc:(c + 1) * eloc])
        m["idx"] = iw
        m["ha1"] = np.ascontiguousarray(ha1[:, c * nloc:(c + 1) * nloc])
        in_maps.append(m)
    return in_maps


def kernel(**inputs):
    key = "full"
    if key not in _CACHE:
        _CACHE[key] = _build()
    nc = _CACHE[key]
    in_maps = _prep_inputs(**inputs)
    res = run_bass_kernel_spmd(nc, in_maps, core_ids=list(range(NCORES)))
    return np.concatenate([res.results[c]["out"] for c in range(NCORES)], axis=0)


# revision 11
# speedup vs baseline: 1.0104x; 1.0104x over previous
"""PiGNNLayer Trainium2 Bass kernel.

Computes the reference nn_PiGNNLayer graph-attention layer on 8 NeuronCores.

Sharding: core c owns nodes [c*N/8, (c+1)*N/8) and their contiguous K=30-edge
blocks.  All MLPs / softmax / weighted sums are local to a node's edge block;
the scatter-mean for the gate needs one 512-byte AllReduce.

On-chip layout is feature-major: activations live as [128 features, edges]
tiles so every Linear layer is a weights-stationary matmul
(out_T = W.T @ x_T via matmul(out, lhsT=W, rhs=x_T)).  Edge features e are
pre-transposed on the host; h_dst rows are gathered on-device with
dma_gather(transpose=True) (bf16, 16-bit-granular transpose) straight into
feature-major tiles.  h_src is the node's own row repeated K times, so its
first-layer contribution (h @ aw1[:128] + ab1) is precomputed per node on the
host and added into the layer-1 PSUM with an identity matmul whose rhs uses a
stride-0 column-repeat access pattern.

Each MLP stage accumulates into a 4-bank PSUM tile [128, 4, 512] (one 480-col
node-aligned matmul per bank) and is evacuated to SBUF by a single wide
ACT/DVE instruction reading the strided [128, 4, 480] view, which amortizes
the ~352-cycle per-instruction engine overhead.

Softmax over the K=30 neighbors skips the max-subtraction (logits are O(1) by
construction), exponentiates attention logits that were replicated across all
128 partitions by tiling the aw3 column 128x, and defers normalization to
after the attention-weighted K-sum.  node_mlp layer 3 and to_h are fused into
a single node-level matmul with W' = nw3 @ thw (host-precomputed) because the
einsum commutes with the last linear layer (and sum(att)=1 handles the bias).
"""

import sys
import os

for _p in ("/opt/trn_rl_repo",):
    if _p not in sys.path and os.path.isdir(_p):
        sys.path.insert(0, _p)

import numpy as np
import ml_dtypes
from contextlib import ExitStack

import concourse.bass as bass
import concourse.bacc as bacc
import concourse.tile as tile
import concourse.mybir as mybir
from concourse.bass_utils import run_bass_kernel_spmd
from concourse.library_config import mlp as _mlp_lib

AF = mybir.ActivationFunctionType
OP = mybir.AluOpType
F32 = mybir.dt.float32
F32R = mybir.dt.float32r
BF16 = mybir.dt.bfloat16
I16 = mybir.dt.int16

# Problem shape (hardcoded per spec).
N, K, D, H = 16384, 30, 128, 1
NCORES = 8
NLOC = N // NCORES            # nodes per core
ELOC = NLOC * K               # edges per core
BLKN = 16                     # nodes per matmul block
BLK = BLKN * K                # 480 free-dim columns per matmul block
SCN = 64                      # nodes per super-chunk
SCC = SCN * K                 # 1920 columns per super-chunk
NBLK = SCC // BLK             # 4 blocks per super-chunk
NSC = NLOC // SCN             # super-chunks per core
BATCH = 4                     # super-chunks per att/node phase batch
GC = 640                      # idxs per dma_gather call (HW limit <= 768)
SCALE = 1.0 / float(np.sqrt(D // H))

_CACHE = {}


def _build(ncores=NCORES, nloc=NLOC, use_collective=True, use_gather=True):
    nsc = nloc // SCN
    nbatch = nsc // BATCH
    eloc = nloc * K
    nc = bacc.Bacc("TRN2", target_bir_lowering=False, debug=False,
                   num_devices=ncores)

    def din(name, shape, dt):
        return nc.dram_tensor(name, shape, dt, kind="ExternalInput").ap()

    eT = din("eT", [128, eloc], F32R)
    hb = din("hb", [nloc * ncores, 128], BF16)
    idx = din("idx", [128, eloc // 16], I16)
    ha1 = din("ha1", [128, nloc], F32R)
    aw1e = din("aw1e", [128, 128], F32R)
    aw1h = din("aw1h", [128, 128], BF16)
    ident = din("identw", [128, 128], F32R)
    identf = din("identf", [128, 128], F32)
    aw2 = din("aw2", [128, 128], F32R)
    aw3r = din("aw3r", [128, 128], F32R)
    nw1e = din("nw1e", [128, 128], F32R)
    nw1h = din("nw1h", [128, 128], BF16)
    nw2 = din("nw2", [128, 128], F32R)
    wp = din("wp", [128, 128], F32R)
    gw1 = din("gw1", [128, 128], F32)
    gw2 = din("gw2", [128, 128], F32)
    gw3 = din("gw3", [128, 128], F32)
    # bias vectors packed [128, 7]: ab2, nb1, nb2, bp, gb1, gb2, gb3/2
    bvec = din("bvec", [128, 7], F32)
    out = nc.dram_tensor("out", [nloc, 128], F32, kind="ExternalOutput").ap()

    with tile.TileContext(nc) as tc, ExitStack() as ctx:
        wpool = ctx.enter_context(tc.tile_pool(name="wpool", bufs=1))
        per = ctx.enter_context(tc.tile_pool(name="per", bufs=1))
        epool = ctx.enter_context(tc.tile_pool(name="epool", bufs=BATCH + 1))
        jpool = ctx.enter_context(tc.tile_pool(name="jpool", bufs=BATCH + 1))
        ppool = ctx.enter_context(tc.tile_pool(name="ppool", bufs=BATCH))
        xpool = ctx.enter_context(tc.tile_pool(name="xpool", bufs=3))
        y2pool = ctx.enter_context(tc.tile_pool(name="y2pool", bufs=2))
        opool = ctx.enter_context(tc.tile_pool(name="opool", bufs=2))
        psG = ctx.enter_context(tc.tile_pool(name="psG", bufs=4, space="PSUM"))
        dram = ctx.enter_context(tc.tile_pool(name="dram", bufs=1, space="DRAM"))

        # --- load weights / persistent data -------------------------------
        def wtile(src, dt):
            t = wpool.tile([128, src.shape[1]], dt, name=f"w_{src.tensor.name}")
            nc.sync.dma_start(t[:], src[:])
            return t

        w_aw1e = wtile(aw1e, F32R)
        w_aw1h = wtile(aw1h, BF16)
        w_ident = wtile(ident, F32R)
        w_identf = wtile(identf, F32)
        w_aw2 = wtile(aw2, F32R)
        w_aw3r = wtile(aw3r, F32R)
        w_nw1e = wtile(nw1e, F32R)
        w_nw1h = wtile(nw1h, BF16)
        w_nw2 = wtile(nw2, F32R)
        w_wp = wtile(wp, F32R)
        w_gw1 = wtile(gw1, F32)
        w_gw2 = wtile(gw2, F32)
        w_gw3 = wtile(gw3, F32)
        w_bias = wtile(bvec, F32)
        b_ab2 = w_bias[:, 0:1]
        b_nb1 = w_bias[:, 1:2]
        b_nb2 = w_bias[:, 2:3]
        b_bp = w_bias[:, 3:4]
        b_gb1 = w_bias[:, 4:5]
        b_gb2 = w_bias[:, 5:6]
        b_gb3h = w_bias[:, 6:7]

        ha1_sb = per.tile([128, nloc], F32R)
        nc.sync.dma_start(ha1_sb[:], ha1[:])
        idx_sb = per.tile([128, eloc // 16], I16)
        nc.sync.dma_start(idx_sb[:], idx[:])
        u_sb = per.tile([128, nloc], F32)
        den_sb = per.tile([128, nloc], F32)
        hh_sb = per.tile([128, nloc], F32)

        nc.gpsimd.load_library(_mlp_lib)

        def stage_psum(name):
            # a stage is a pair of 2-bank tiles (4 blocks of 480 total)
            return [psG.tile([128, 2, 512], F32, name=f"{name}_h{h}", tag="G")
                    for h in range(2)]

        def mm_blk(ps, b, w, rhs, start, stop):
            nc.tensor.matmul(ps[b // 2][:, b % 2, 0:BLK], w[:], rhs,
                             start=start, stop=stop)

        def half(ps, h):
            return ps[h][:, :, 0:BLK]

        def grouped(t, n):
            return t[:].rearrange("p (n k) -> p n k", n=n)

        # --- main loop ----------------------------------------------------
        for bi in range(nbatch):
            e_tiles, j_tiles, p_tiles = [], [], []
            for sj in range(BATCH):
                sc = bi * BATCH + sj
                c0 = sc * SCC
                et = epool.tile([128, SCC], F32R, name=f"et{bi}_{sj}", tag="et")
                nc.sync.dma_start(et[:], eT[:, c0:c0 + SCC])
                jt = jpool.tile([128, SCC], BF16, name=f"jt{bi}_{sj}", tag="jt")
                if use_gather:
                    # transpose-mode dma_gather is limited to <=768 idxs/call
                    for gj in range(SCC // GC):
                        i0 = sc * (SCC // 16) + gj * (GC // 16)
                        nc.gpsimd.dma_gather(
                            jt[:, gj * GC:(gj + 1) * GC].unsqueeze(1), hb[:],
                            idx_sb[:, i0:i0 + GC // 16],
                            GC, GC, 128, transpose=True)
                else:
                    nc.gpsimd.memset(jt[:], 1.0)
                e_tiles.append(et)
                j_tiles.append(jt)

            # ---- attention phase over the batch --------------------------
            for sj in range(BATCH):
                sc = bi * BATCH + sj
                n0 = sc * SCN
                et, jt = e_tiles[sj], j_tiles[sj]
                pt = ppool.tile([128, SCC], BF16, name=f"pt{bi}_{sj}", tag="pt")
                p_tiles.append(pt)

                a1 = stage_psum(f"a1_{sc}")
                for b in range(NBLK):
                    mm_blk(a1, b, w_aw1e, et[:, b * BLK:(b + 1) * BLK], True, False)
                for b in range(NBLK):
                    mm_blk(a1, b, w_aw1h, jt[:, b * BLK:(b + 1) * BLK], False, False)
                for b in range(NBLK):
                    rep = ha1_sb[:, n0 + b * BLKN:n0 + (b + 1) * BLKN] \
                        .unsqueeze(2).broadcast_to([128, BLKN, K])
                    nc.tensor.matmul(
                        a1[b // 2][:, b % 2, 0:BLK].rearrange(
                            "p (n k) -> p n k", n=BLKN),
                        w_ident[:], rep, start=False, stop=True)

                x1 = xpool.tile([128, SCC], F32R, name=f"x1_{sc}", tag="x")
                for h in range(2):
                    xv = x1[:, h * 2 * BLK:(h + 1) * 2 * BLK] \
                        .rearrange("p (b c) -> p b c", b=2)
                    nc.vector.tensor_scalar_max(xv, half(a1, h), 0.0)

                a2 = stage_psum(f"a2_{sc}")
                for b in range(NBLK):
                    mm_blk(a2, b, w_aw2, x1[:, b * BLK:(b + 1) * BLK], True, True)
                x2 = xpool.tile([128, SCC], F32R, name=f"x2_{sc}", tag="x")
                for h in range(2):
                    xv = x2[:, h * 2 * BLK:(h + 1) * 2 * BLK] \
                        .rearrange("p (b c) -> p b c", b=2)
                    if h == 0 and sc % 2 == 0:
                        nc.scalar.activation(xv, half(a2, h), AF.Relu, bias=b_ab2)
                    else:
                        nc.vector.tensor_scalar(xv, half(a2, h), b_ab2, 0.0,
                                                OP.add, OP.max)

                a3 = stage_psum(f"a3_{sc}")
                for b in range(NBLK):
                    mm_blk(a3, b, w_aw3r, x2[:, b * BLK:(b + 1) * BLK], True, True)
                for h in range(2):
                    pv = pt[:, h * 2 * BLK:(h + 1) * 2 * BLK] \
                        .rearrange("p (b c) -> p b c", b=2)
                    nc.scalar.activation(pv, half(a3, h), AF.Exp, scale=SCALE)

                nc.vector.reduce_sum(den_sb[:, n0:n0 + SCN], grouped(pt, SCN),
                                     axis=mybir.AxisListType.X)

            # ---- node-value phase over the batch -------------------------
            for sj in range(BATCH):
                sc = bi * BATCH + sj
                n0 = sc * SCN
                et, jt, pt = e_tiles[sj], j_tiles[sj], p_tiles[sj]

                n1 = stage_psum(f"n1_{sc}")
                for b in range(NBLK):
                    mm_blk(n1, b, w_nw1e, et[:, b * BLK:(b + 1) * BLK], True, False)
                for b in range(NBLK):
                    mm_blk(n1, b, w_nw1h, jt[:, b * BLK:(b + 1) * BLK], False, True)
                y1 = xpool.tile([128, SCC], F32R, name=f"y1_{sc}", tag="x")
                for h in range(2):
                    yv = y1[:, h * 2 * BLK:(h + 1) * 2 * BLK] \
                        .rearrange("p (b c) -> p b c", b=2)
                    nc.scalar.activation(yv, half(n1, h), AF.Gelu, bias=b_nb1)

                n2 = stage_psum(f"n2_{sc}")
                for b in range(NBLK):
                    mm_blk(n2, b, w_nw2, y1[:, b * BLK:(b + 1) * BLK], True, True)
                y2 = y2pool.tile([128, SCC], BF16, name=f"y2_{sc}", tag="y2")
                for h in range(2):
                    yv = y2[:, h * 2 * BLK:(h + 1) * 2 * BLK] \
                        .rearrange("p (b c) -> p b c", b=2)
                    nc.scalar.activation(yv, half(n2, h), AF.Gelu, bias=b_nb2)

                # p <- p * y2, then K-group sum into u
                nc.gpsimd.tensor_tensor(pt[:], y2[:], pt[:], OP.mult)
                nc.vector.reduce_sum(u_sb[:, n0:n0 + SCN], grouped(pt, SCN),
                                     axis=mybir.AxisListType.X)

        # --- tail: normalize, project, gate, transpose, store -------------
        deni = per.tile([128, nloc], F32)
        nc.vector.reciprocal(deni[:], den_sb[:])
        ubar = per.tile([128, nloc], F32R)
        nc.vector.tensor_tensor(ubar[:], u_sb[:], deni[:], OP.mult)

        nhb = nloc // 512
        hps = stage_psum("hps")
        for b in range(nhb):
            nc.tensor.matmul(hps[b // 2][:, b % 2, :], w_wp[:],
                             ubar[:, b * 512:(b + 1) * 512],
                             start=True, stop=True)
        for h in range((nhb + 1) // 2):
            w = min(2, nhb - h * 2)
            nc.scalar.activation(
                hh_sb[:, h * 1024:h * 1024 + w * 512]
                .rearrange("p (b c) -> p b c", b=w),
                hps[h][:, 0:w, :], AF.Identity, bias=b_bp)

        csum = per.tile([128, 1], F32)
        nc.vector.reduce_sum(csum[:], hh_sb[:], axis=mybir.AxisListType.X)

        cin = dram.tile([128, 1], F32)
        cout = dram.tile([128, 1], F32,
                         addr_space="Shared" if ncores > 4 else "Local")
        nc.sync.dma_start(cin[:], csum[:])
        if use_collective:
            nc.gpsimd.collective_compute(
                "AllReduce", OP.add,
                replica_groups=[list(range(ncores))],
                ins=[cin[:].opt()], outs=[cout[:].opt()])
        else:
            nc.sync.dma_start(cout[:], cin[:])
        call = per.tile([128, 1], F32)
        nc.sync.dma_start(call[:], cout[:])

        # gate MLP (redundant on every core), sigmoid via tanh
        gps = stage_psum("gps")
        nc.tensor.matmul(gps[0][:, 0, 0:1], w_gw1[:], call[:], start=True, stop=True)
        g1 = per.tile([128, 1], F32)
        nc.scalar.activation(g1[:], gps[0][:, 0, 0:1], AF.Relu, bias=b_gb1,
                             scale=1.0 / float(ncores * nloc))
        nc.tensor.matmul(gps[0][:, 1, 0:1], w_gw2[:], g1[:], start=True, stop=True)
        g2 = per.tile([128, 1], F32)
        nc.scalar.activation(g2[:], gps[0][:, 1, 0:1], AF.Relu, bias=b_gb2)
        nc.tensor.matmul(gps[1][:, 0, 0:1], w_gw3[:], g2[:], start=True, stop=True)
        gth = per.tile([128, 1], F32)
        nc.scalar.activation(gth[:], gps[1][:, 0, 0:1], AF.Tanh, bias=b_gb3h,
                             scale=0.5)
        gv = per.tile([128, 1], F32)
        nc.vector.tensor_scalar(gv[:], gth[:], 0.5, 0.5, OP.mult, OP.add)

        nc.vector.tensor_scalar(hh_sb[:], hh_sb[:], gv[:], None, OP.mult)

        # transpose [128 feat, nloc] -> [nloc, 128] and store
        for q in range(nloc // 512):
            tps = stage_psum(f"tps_{q}")
            for b in range(4):
                nc.tensor.transpose(tps[b // 2][:, b % 2, 0:128],
                                    hh_sb[:, q * 512 + b * 128:q * 512 + (b + 1) * 128],
                                    w_identf[:])
            ot = opool.tile([128, 4, 128], F32, name=f"ot_{q}", tag="ot")
            for h in range(2):
                nc.vector.tensor_copy(ot[:, h * 2:(h + 1) * 2, :],
                                      tps[h][:, :, 0:128])
            # DRAM rows q*512 + b*128 + p, col f  <-  ot[p, b, f]
            dview = out[q * 512:(q + 1) * 512, :] \
                .rearrange("(b p) f -> p b f", b=4)
            nc.sync.dma_start(dview, ot[:])

    nc.compile()
    return nc


def _prep_inputs(h, e, aw1, ab1, aw2, ab2, aw3, ab3,
                 nw1, nb1, nw2, nb2, nw3, nb3, thw,
                 gw1, gb1, gw2, gb2, gw3, gb3,
                 edge_idx, batch_idx, ncores=NCORES, nloc=NLOC):
    n = ncores * nloc
    eloc = nloc * K
    src = np.asarray(edge_idx[0])
    assert np.array_equal(src, np.repeat(np.arange(n, dtype=src.dtype), K)), \
        "kernel assumes edge_idx[0] == repeat(arange(N), K)"
    assert np.all(np.asarray(batch_idx) == 0), "kernel assumes batch_idx == 0"
    dst = np.asarray(edge_idx[1]).astype(np.int16)

    h = np.asarray(h, np.float32)
    e = np.asarray(e, np.float32)
    eT = np.ascontiguousarray(e.T)                      # [128, E]
    hb = np.ascontiguousarray(h.astype(ml_dtypes.bfloat16))
    ha1 = np.ascontiguousarray((h @ np.asarray(aw1)[:D] + np.asarray(ab1)).T)
    wp = np.asarray(nw3, np.float32) @ np.asarray(thw, np.float32)
    bp = np.asarray(nb3, np.float32) @ np.asarray(thw, np.float32)
    aw3r = np.ascontiguousarray(np.tile(np.asarray(aw3, np.float32), (1, 128)))
    identw = np.eye(128, dtype=np.float32)

    bvec = np.stack([
        np.asarray(ab2, np.float32), np.asarray(nb1, np.float32),
        np.asarray(nb2, np.float32), bp,
        np.asarray(gb1, np.float32), np.asarray(gb2, np.float32),
        np.asarray(gb3, np.float32) * 0.5,
    ], axis=1)                                          # [128, 7]

    common = {
        "hb": hb,
        "aw1e": np.ascontiguousarray(np.asarray(aw1, np.float32)[D:2 * D]),
        "aw1h": np.ascontiguousarray(
            np.asarray(aw1, np.float32)[2 * D:3 * D].astype(ml_dtypes.bfloat16)),
        "identw": identw, "identf": identw,
        "aw2": np.asarray(aw2, np.float32),
        "aw3r": aw3r,
        "nw1e": np.ascontiguousarray(np.asarray(nw1, np.float32)[:D]),
        "nw1h": np.ascontiguousarray(
            np.asarray(nw1, np.float32)[D:2 * D].astype(ml_dtypes.bfloat16)),
        "nw2": np.asarray(nw2, np.float32),
        "wp": wp,
        "gw1": np.asarray(gw1, np.float32),
        "gw2": np.asarray(gw2, np.float32),
        "gw3": np.asarray(gw3, np.float32),
        "bvec": np.ascontiguousarray(bvec),
    }

    in_maps = []
    for c in range(ncores):
        dc = dst[c * eloc:(c + 1) * eloc]
        iw = np.ascontiguousarray(np.tile(dc.reshape(-1, 16).T, (8, 1)))
        m = dict(common)
        m["eT"] = np.ascontiguousarray(eT[:, c * eloc:(c + 1) * eloc])
        m["idx"] = iw
        m["ha1"] = np.ascontiguousarray(ha1[:, c * nloc:(c + 1) * nloc])
        in_maps.append(m)
    return in_maps


def kernel(**inputs):
    key = "full"
    if key not in _CACHE:
        _CACHE[key] = _build()
    nc = _CACHE[key]
    in_maps = _prep_inputs(**inputs)
    res = run_bass_kernel_spmd(nc, in_maps, core_ids=list(range(NCORES)))
    return np.concatenate([res.results[c]["out"] for c in range(NCORES)], axis=0)


# revision 13
# speedup vs baseline: 10240.2750x; 10134.4928x over previous
"""PiGNNLayer Trainium2 Bass kernel.

Computes the reference nn_PiGNNLayer graph-attention layer on 8 NeuronCores.

Sharding: core c owns nodes [c*N/8, (c+1)*N/8) and their contiguous K=30-edge
blocks.  All MLPs / softmax / weighted sums are local to a node's edge block;
the scatter-mean for the gate needs one 512-byte AllReduce.

On-chip layout is feature-major: activations live as [128 features, edges]
tiles so every Linear layer is a weights-stationary matmul
(out_T = W.T @ x_T via matmul(out, lhsT=W, rhs=x_T)).  Edge features e are
pre-transposed on the host; h_dst rows are gathered on-device with
dma_gather(transpose=True) (bf16, 16-bit-granular transpose) straight into
feature-major tiles.  h_src is the node's own row repeated K times, so its
first-layer contribution (h @ aw1[:128] + ab1) is precomputed per node on the
host and added into the layer-1 PSUM with an identity matmul whose rhs uses a
stride-0 column-repeat access pattern.

Each MLP stage accumulates into a 4-bank PSUM tile [128, 4, 512] (one 480-col
node-aligned matmul per bank) and is evacuated to SBUF by a single wide
ACT/DVE instruction reading the strided [128, 4, 480] view, which amortizes
the ~352-cycle per-instruction engine overhead.

Softmax over the K=30 neighbors skips the max-subtraction (logits are O(1) by
construction), exponentiates attention logits that were replicated across all
128 partitions by tiling the aw3 column 128x, and defers normalization to
after the attention-weighted K-sum.  node_mlp layer 3 and to_h are fused into
a single node-level matmul with W' = nw3 @ thw (host-precomputed) because the
einsum commutes with the last linear layer (and sum(att)=1 handles the bias).
"""

import sys
import os

for _p in ("/opt/trn_rl_repo",):
    if _p not in sys.path and os.path.isdir(_p):
        sys.path.insert(0, _p)

import numpy as np
import ml_dtypes
from contextlib import ExitStack

import concourse.bass as bass
import concourse.bacc as bacc
import concourse.tile as tile
import concourse.mybir as mybir
from concourse.bass_utils import run_bass_kernel_spmd
from concourse.library_config import mlp as _mlp_lib

AF = mybir.ActivationFunctionType
OP = mybir.AluOpType
F32 = mybir.dt.float32
F32R = mybir.dt.float32r
BF16 = mybir.dt.bfloat16
I16 = mybir.dt.int16

# Problem shape (hardcoded per spec).
N, K, D, H = 16384, 30, 128, 1
NCORES = 8
NLOC = N // NCORES            # nodes per core
ELOC = NLOC * K               # edges per core
BLKN = 16                     # nodes per matmul block
BLK = BLKN * K                # 480 free-dim columns per matmul block
SCN = 64                      # nodes per super-chunk
SCC = SCN * K                 # 1920 columns per super-chunk
NBLK = SCC // BLK             # 4 blocks per super-chunk
NSC = NLOC // SCN             # super-chunks per core
BATCH = 4                     # super-chunks per att/node phase batch
GC = 640                      # idxs per dma_gather call (HW limit <= 768)
SCALE = 1.0 / float(np.sqrt(D // H))

_CACHE = {}


def _build(ncores=NCORES, nloc=NLOC, use_collective=True, use_gather=True):
    nsc = nloc // SCN
    nbatch = nsc // BATCH
    eloc = nloc * K
    nc = bacc.Bacc("TRN2", target_bir_lowering=False, debug=False,
                   num_devices=ncores)

    def din(name, shape, dt):
        return nc.dram_tensor(name, shape, dt, kind="ExternalInput").ap()

    eT = din("eT", [128, eloc], F32R)
    hb = din("hb", [nloc * ncores, 128], BF16)
    idx = din("idx", [128, eloc // 16], I16)
    ha1 = din("ha1", [128, nloc], F32R)
    aw1e = din("aw1e", [128, 128], F32R)
    aw1h = din("aw1h", [128, 128], BF16)
    ident = din("identw", [128, 128], F32R)
    identf = din("identf", [128, 128], F32)
    aw2 = din("aw2", [128, 128], F32R)
    aw3r = din("aw3r", [128, 128], F32R)
    nw1e = din("nw1e", [128, 128], F32R)
    nw1h = din("nw1h", [128, 128], BF16)
    nw2 = din("nw2", [128, 128], F32R)
    wp = din("wp", [128, 128], F32R)
    gw1 = din("gw1", [128, 128], F32)
    gw2 = din("gw2", [128, 128], F32)
    gw3 = din("gw3", [128, 128], F32)
    # bias vectors packed [128, 7]: ab2, nb1, nb2, bp, gb1, gb2, gb3/2
    bvec = din("bvec", [128, 7], F32)
    out = nc.dram_tensor("out", [nloc, 128], F32, kind="ExternalOutput").ap()

    with tile.TileContext(nc) as tc, ExitStack() as ctx:
        wpool = ctx.enter_context(tc.tile_pool(name="wpool", bufs=1))
        per = ctx.enter_context(tc.tile_pool(name="per", bufs=1))
        epool = ctx.enter_context(tc.tile_pool(name="epool", bufs=BATCH + 1))
        jpool = ctx.enter_context(tc.tile_pool(name="jpool", bufs=BATCH + 1))
        ppool = ctx.enter_context(tc.tile_pool(name="ppool", bufs=BATCH + 1))
        xpool = ctx.enter_context(tc.tile_pool(name="xpool", bufs=3))
        y2pool = ctx.enter_context(tc.tile_pool(name="y2pool", bufs=3))
        opool = ctx.enter_context(tc.tile_pool(name="opool", bufs=2))
        psG = ctx.enter_context(tc.tile_pool(name="psG", bufs=4, space="PSUM"))
        dram = ctx.enter_context(tc.tile_pool(name="dram", bufs=1, space="DRAM"))

        # --- load weights / persistent data -------------------------------
        def wtile(src, dt):
            t = wpool.tile([128, src.shape[1]], dt, name=f"w_{src.tensor.name}")
            nc.sync.dma_start(t[:], src[:])
            return t

        w_aw1e = wtile(aw1e, F32R)
        w_aw1h = wtile(aw1h, BF16)
        w_ident = wtile(ident, F32R)
        w_identf = wtile(identf, F32)
        w_aw2 = wtile(aw2, F32R)
        w_aw3r = wtile(aw3r, F32R)
        w_nw1e = wtile(nw1e, F32R)
        w_nw1h = wtile(nw1h, BF16)
        w_nw2 = wtile(nw2, F32R)
        w_wp = wtile(wp, F32R)
        w_gw1 = wtile(gw1, F32)
        w_gw2 = wtile(gw2, F32)
        w_gw3 = wtile(gw3, F32)
        w_bias = wtile(bvec, F32)
        b_ab2 = w_bias[:, 0:1]
        b_nb1 = w_bias[:, 1:2]
        b_nb2 = w_bias[:, 2:3]
        b_bp = w_bias[:, 3:4]
        b_gb1 = w_bias[:, 4:5]
        b_gb2 = w_bias[:, 5:6]
        b_gb3h = w_bias[:, 6:7]

        ha1_sb = per.tile([128, nloc], F32R)
        nc.sync.dma_start(ha1_sb[:], ha1[:])
        idx_sb = per.tile([128, eloc // 16], I16)
        nc.sync.dma_start(idx_sb[:], idx[:])
        u_sb = per.tile([128, nloc], F32)
        den_sb = per.tile([128, nloc], F32)
        hh_sb = per.tile([128, nloc], F32)

        nc.gpsimd.load_library(_mlp_lib)

        def stage_psum(name):
            # a stage is a pair of 2-bank tiles (4 blocks of 480 total)
            return [psG.tile([128, 2, 512], F32, name=f"{name}_h{h}", tag="G")
                    for h in range(2)]

        def mm_blk(ps, b, w, rhs, start, stop):
            nc.tensor.matmul(ps[b // 2][:, b % 2, 0:BLK], w[:], rhs,
                             start=start, stop=stop)

        def half(ps, h):
            return ps[h][:, :, 0:BLK]

        def grouped(t, n):
            return t[:].rearrange("p (n k) -> p n k", n=n)

        # --- main loop ----------------------------------------------------
        for bi in range(nbatch):
            e_tiles, j_tiles, p_tiles = [], [], []
            for sj in range(BATCH):
                sc = bi * BATCH + sj
                c0 = sc * SCC
                et = epool.tile([128, SCC], F32R, name=f"et{bi}_{sj}", tag="et")
                nc.sync.dma_start(et[:], eT[:, c0:c0 + SCC])
                jt = jpool.tile([128, SCC], BF16, name=f"jt{bi}_{sj}", tag="jt")
                if use_gather:
                    # transpose-mode dma_gather is limited to <=768 idxs/call
                    for gj in range(SCC // GC):
                        i0 = sc * (SCC // 16) + gj * (GC // 16)
                        nc.gpsimd.dma_gather(
                            jt[:, gj * GC:(gj + 1) * GC].unsqueeze(1), hb[:],
                            idx_sb[:, i0:i0 + GC // 16],
                            GC, GC, 128, transpose=True)
                else:
                    nc.gpsimd.memset(jt[:], 1.0)
                e_tiles.append(et)
                j_tiles.append(jt)

            # ---- attention phase over the batch --------------------------
            for sj in range(BATCH):
                sc = bi * BATCH + sj
                n0 = sc * SCN
                et, jt = e_tiles[sj], j_tiles[sj]
                pt = ppool.tile([128, SCC], BF16, name=f"pt{bi}_{sj}", tag="pt")
                p_tiles.append(pt)

                a1 = stage_psum(f"a1_{sc}")
                for b in range(NBLK):
                    mm_blk(a1, b, w_aw1e, et[:, b * BLK:(b + 1) * BLK], True, False)
                for b in range(NBLK):
                    mm_blk(a1, b, w_aw1h, jt[:, b * BLK:(b + 1) * BLK], False, False)
                for b in range(NBLK):
                    rep = ha1_sb[:, n0 + b * BLKN:n0 + (b + 1) * BLKN] \
                        .unsqueeze(2).broadcast_to([128, BLKN, K])
                    nc.tensor.matmul(
                        a1[b // 2][:, b % 2, 0:BLK].rearrange(
                            "p (n k) -> p n k", n=BLKN),
                        w_ident[:], rep, start=False, stop=True)

                x1 = xpool.tile([128, SCC], F32R, name=f"x1_{sc}", tag="x")
                for h in range(2):
                    xv = x1[:, h * 2 * BLK:(h + 1) * 2 * BLK] \
                        .rearrange("p (b c) -> p b c", b=2)
                    if h == 1 and sc % 2 == 1:
                        nc.scalar.activation(xv, half(a1, h), AF.Relu)
                    else:
                        nc.vector.tensor_scalar_max(xv, half(a1, h), 0.0)

                a2 = stage_psum(f"a2_{sc}")
                for b in range(NBLK):
                    mm_blk(a2, b, w_aw2, x1[:, b * BLK:(b + 1) * BLK], True, True)
                x2 = xpool.tile([128, SCC], F32R, name=f"x2_{sc}", tag="x")
                for h in range(2):
                    xv = x2[:, h * 2 * BLK:(h + 1) * 2 * BLK] \
                        .rearrange("p (b c) -> p b c", b=2)
                    if h == 0 and sc % 2 == 0:
                        nc.scalar.activation(xv, half(a2, h), AF.Relu, bias=b_ab2)
                    else:
                        nc.vector.tensor_scalar(xv, half(a2, h), b_ab2, 0.0,
                                                OP.add, OP.max)

                a3 = stage_psum(f"a3_{sc}")
                for b in range(NBLK):
                    mm_blk(a3, b, w_aw3r, x2[:, b * BLK:(b + 1) * BLK], True, True)
                for h in range(2):
                    pv = pt[:, h * 2 * BLK:(h + 1) * 2 * BLK] \
                        .rearrange("p (b c) -> p b c", b=2)
                    nc.scalar.activation(pv, half(a3, h), AF.Exp, scale=SCALE)

                nc.vector.reduce_sum(den_sb[:, n0:n0 + SCN], grouped(pt, SCN),
                                     axis=mybir.AxisListType.X)

            # ---- node-value phase over the batch -------------------------
            for sj in range(BATCH):
                sc = bi * BATCH + sj
                n0 = sc * SCN
                et, jt, pt = e_tiles[sj], j_tiles[sj], p_tiles[sj]

                n1 = stage_psum(f"n1_{sc}")
                for b in range(NBLK):
                    mm_blk(n1, b, w_nw1e, et[:, b * BLK:(b + 1) * BLK], True, False)
                for b in range(NBLK):
                    mm_blk(n1, b, w_nw1h, jt[:, b * BLK:(b + 1) * BLK], False, True)
                y1 = xpool.tile([128, SCC], F32R, name=f"y1_{sc}", tag="x")
                for h in range(2):
                    yv = y1[:, h * 2 * BLK:(h + 1) * 2 * BLK] \
                        .rearrange("p (b c) -> p b c", b=2)
                    nc.scalar.activation(yv, half(n1, h), AF.Gelu, bias=b_nb1)

                n2 = stage_psum(f"n2_{sc}")
                for b in range(NBLK):
                    mm_blk(n2, b, w_nw2, y1[:, b * BLK:(b + 1) * BLK], True, True)
                y2 = y2pool.tile([128, SCC], BF16, name=f"y2_{sc}", tag="y2")
                for h in range(2):
                    yv = y2[:, h * 2 * BLK:(h + 1) * 2 * BLK] \
                        .rearrange("p (b c) -> p b c", b=2)
                    nc.scalar.activation(yv, half(n2, h), AF.Gelu, bias=b_nb2)

                # p <- p * y2, then K-group sum into u
                eng = nc.gpsimd if sc % 2 == 0 else nc.vector
                eng.tensor_tensor(pt[:], y2[:], pt[:], OP.mult)
                nc.vector.reduce_sum(u_sb[:, n0:n0 + SCN], grouped(pt, SCN),
                                     axis=mybir.AxisListType.X)

        # --- tail: normalize, project, gate, transpose, store -------------
        deni = per.tile([128, nloc], F32)
        nc.vector.reciprocal(deni[:], den_sb[:])
        ubar = per.tile([128, nloc], F32R)
        nc.vector.tensor_tensor(ubar[:], u_sb[:], deni[:], OP.mult)

        nhb = nloc // 512
        hps = stage_psum("hps")
        for b in range(nhb):
            nc.tensor.matmul(hps[b // 2][:, b % 2, :], w_wp[:],
                             ubar[:, b * 512:(b + 1) * 512],
                             start=True, stop=True)
        for h in range((nhb + 1) // 2):
            w = min(2, nhb - h * 2)
            nc.scalar.activation(
                hh_sb[:, h * 1024:h * 1024 + w * 512]
                .rearrange("p (b c) -> p b c", b=w),
                hps[h][:, 0:w, :], AF.Identity, bias=b_bp)

        csum = per.tile([128, 1], F32)
        nc.vector.reduce_sum(csum[:], hh_sb[:], axis=mybir.AxisListType.X)

        cin = dram.tile([128, 1], F32)
        cout = dram.tile([128, 1], F32,
                         addr_space="Shared" if ncores > 4 else "Local")
        nc.sync.dma_start(cin[:], csum[:])
        if use_collective:
            nc.gpsimd.collective_compute(
                "AllReduce", OP.add,
                replica_groups=[list(range(ncores))],
                ins=[cin[:].opt()], outs=[cout[:].opt()])
        else:
            nc.sync.dma_start(cout[:], cin[:])
        call = per.tile([128, 1], F32)
        nc.sync.dma_start(call[:], cout[:])

        # gate MLP (redundant on every core), sigmoid via tanh
        gps = stage_psum("gps")
        nc.tensor.matmul(gps[0][:, 0, 0:1], w_gw1[:], call[:], start=True, stop=True)
        g1 = per.tile([128, 1], F32)
        nc.scalar.activation(g1[:], gps[0][:, 0, 0:1], AF.Relu, bias=b_gb1,
                             scale=1.0 / float(ncores * nloc))
        nc.tensor.matmul(gps[0][:, 1, 0:1], w_gw2[:], g1[:], start=True, stop=True)
        g2 = per.tile([128, 1], F32)
        nc.scalar.activation(g2[:], gps[0][:, 1, 0:1], AF.Relu, bias=b_gb2)
        nc.tensor.matmul(gps[1][:, 0, 0:1], w_gw3[:], g2[:], start=True, stop=True)
        gth = per.tile([128, 1], F32)
        nc.scalar.activation(gth[:], gps[1][:, 0, 0:1], AF.Tanh, bias=b_gb3h,
                             scale=0.5)
        gv = per.tile([128, 1], F32)
        nc.vector.tensor_scalar(gv[:], gth[:], 0.5, 0.5, OP.mult, OP.add)

        nc.vector.tensor_scalar(hh_sb[:], hh_sb[:], gv[:], None, OP.mult)

        # transpose [128 feat, nloc] -> [nloc, 128] and store
        for q in range(nloc // 512):
            tps = stage_psum(f"tps_{q}")
            for b in range(4):
                nc.tensor.transpose(tps[b // 2][:, b % 2, 0:128],
                                    hh_sb[:, q * 512 + b * 128:q * 512 + (b + 1) * 128],
                                    w_identf[:])
            ot = opool.tile([128, 4, 128], F32, name=f"ot_{q}", tag="ot")
            for h in range(2):
                nc.vector.tensor_copy(ot[:, h * 2:(h + 1) * 2, :],
                                      tps[h][:, :, 0:128])
            # DRAM rows q*512 + b*128 + p, col f  <-  ot[p, b, f]
            dview = out[q * 512:(q + 1) * 512, :] \
                .rearrange("(b p) f -> p b f", b=4)
            nc.sync.dma_start(dview, ot[:])

    nc.compile()
    return nc


def _prep_inputs(h, e, aw1, ab1, aw2, ab2, aw3, ab3,
                 nw1, nb1, nw2, nb2, nw3, nb3, thw,
                 gw1, gb1, gw2, gb2, gw3, gb3,
                 edge_idx, batch_idx, ncores=NCORES, nloc=NLOC):
    n = ncores * nloc
    eloc = nloc * K
    src = np.asarray(edge_idx[0])
    assert np.array_equal(src, np.repeat(np.arange(n, dtype=src.dtype), K)), \
        "kernel assumes edge_idx[0] == repeat(arange(N), K)"
    assert np.all(np.asarray(batch_idx) == 0), "kernel assumes batch_idx == 0"
    dst = np.asarray(edge_idx[1]).astype(np.int16)

    h = np.asarray(h, np.float32)
    e = np.asarray(e, np.float32)
    eT = np.ascontiguousarray(e.T)                      # [128, E]
    hb = np.ascontiguousarray(h.astype(ml_dtypes.bfloat16))
    ha1 = np.ascontiguousarray((h @ np.asarray(aw1)[:D] + np.asarray(ab1)).T)
    wp = np.asarray(nw3, np.float32) @ np.asarray(thw, np.float32)
    bp = np.asarray(nb3, np.float32) @ np.asarray(thw, np.float32)
    aw3r = np.ascontiguousarray(np.tile(np.asarray(aw3, np.float32), (1, 128)))
    identw = np.eye(128, dtype=np.float32)

    bvec = np.stack([
        np.asarray(ab2, np.float32), np.asarray(nb1, np.float32),
        np.asarray(nb2, np.float32), bp,
        np.asarray(gb1, np.float32), np.asarray(gb2, np.float32),
        np.asarray(gb3, np.float32) * 0.5,
    ], axis=1)                                          # [128, 7]

    common = {
        "hb": hb,
        "aw1e": np.ascontiguousarray(np.asarray(aw1, np.float32)[D:2 * D]),
        "aw1h": np.ascontiguousarray(
            np.asarray(aw1, np.float32)[2 * D:3 * D].astype(ml_dtypes.bfloat16)),
        "identw": identw, "identf": identw,
        "aw2": np.asarray(aw2, np.float32),
        "aw3r": aw3r,
        "nw1e": np.ascontiguousarray(np.asarray(nw1, np.float32)[:D]),
        "nw1h": np.ascontiguousarray(
            np.asarray(nw1, np.float32)[D:2 * D].astype(ml_dtypes.bfloat16)),
        "nw2": np.asarray(nw2, np.float32),
        "wp": wp,
        "gw1": np.asarray(gw1, np.float32),
        "gw2": np.asarray(gw2, np.float32),
        "gw3": np.asarray(gw3, np.float32),
        "bvec": np.ascontiguousarray(bvec),
    }

    in_maps = []
    for c in range(ncores):
        dc = dst[c * eloc:(c + 1) * eloc]
        iw = np.ascontiguousarray(np.tile(dc.reshape(-1, 16).T, (8, 1)))
        m = dict(common)
        m["eT"] = np.ascontiguousarray(eT[:, c * eloc:(c + 1) * eloc])
        m["idx"] = iw
        m["ha1"] = np.ascontiguousarray(ha1[:, c * nloc:(c + 1) * nloc])
        in_maps.append(m)
    return in_maps


def kernel(**inputs):
    key = "full"
    if key not in _CACHE:
        _CACHE[key] = _build()
    nc = _CACHE[key]
    in_maps = _prep_inputs(**inputs)
    res = run_bass_kernel_spmd(nc, in_maps, core_ids=list(range(NCORES)))
    return np.concatenate([res.results[c]["out"] for c in range(NCORES)], axis=0)


# revision 15
# speedup vs baseline: 10257.4842x; 1.0017x over previous
"""PiGNNLayer Trainium2 Bass kernel.

Computes the reference nn_PiGNNLayer graph-attention layer on 8 NeuronCores.

Sharding: core c owns nodes [c*N/8, (c+1)*N/8) and their contiguous K=30-edge
blocks.  All MLPs / softmax / weighted sums are local to a node's edge block;
the scatter-mean for the gate needs one 512-byte AllReduce.

On-chip layout is feature-major: activations live as [128 features, edges]
tiles so every Linear layer is a weights-stationary matmul
(out_T = W.T @ x_T via matmul(out, lhsT=W, rhs=x_T)).  Edge features e are
pre-transposed on the host; h_dst rows are gathered on-device with
dma_gather(transpose=True) (bf16, 16-bit-granular transpose) straight into
feature-major tiles.  h_src is the node's own row repeated K times, so its
first-layer contribution (h @ aw1[:128] + ab1) is precomputed per node on the
host and added into the layer-1 PSUM with an identity matmul whose rhs uses a
stride-0 column-repeat access pattern.

Each MLP stage accumulates into a 4-bank PSUM tile [128, 4, 512] (one 480-col
node-aligned matmul per bank) and is evacuated to SBUF by a single wide
ACT/DVE instruction reading the strided [128, 4, 480] view, which amortizes
the ~352-cycle per-instruction engine overhead.

Softmax over the K=30 neighbors skips the max-subtraction (logits are O(1) by
construction), exponentiates attention logits that were replicated across all
128 partitions by tiling the aw3 column 128x, and defers normalization to
after the attention-weighted K-sum.  node_mlp layer 3 and to_h are fused into
a single node-level matmul with W' = nw3 @ thw (host-precomputed) because the
einsum commutes with the last linear layer (and sum(att)=1 handles the bias).
"""

import sys
import os

for _p in ("/opt/trn_rl_repo",):
    if _p not in sys.path and os.path.isdir(_p):
        sys.path.insert(0, _p)

import numpy as np
import ml_dtypes
from contextlib import ExitStack

import concourse.bass as bass
import concourse.bacc as bacc
import concourse.tile as tile
import concourse.mybir as mybir
from concourse.bass_utils import run_bass_kernel_spmd
from concourse.library_config import mlp as _mlp_lib

AF = mybir.ActivationFunctionType
OP = mybir.AluOpType
F32 = mybir.dt.float32
F32R = mybir.dt.float32r
BF16 = mybir.dt.bfloat16
I16 = mybir.dt.int16

# Problem shape (hardcoded per spec).
N, K, D, H = 16384, 30, 128, 1
NCORES = 8
NLOC = N // NCORES            # nodes per core
ELOC = NLOC * K               # edges per core
BLKN = 16                     # nodes per matmul block
BLK = BLKN * K                # 480 free-dim columns per matmul block
SCN = 64                      # nodes per super-chunk
SCC = SCN * K                 # 1920 columns per super-chunk
NBLK = SCC // BLK             # 4 blocks per super-chunk
NSC = NLOC // SCN             # super-chunks per core
BATCH = 4                     # super-chunks per att/node phase batch
GC = 640                      # idxs per dma_gather call (HW limit <= 768)
SCALE = 1.0 / float(np.sqrt(D // H))

_CACHE = {}


def _build(ncores=NCORES, nloc=NLOC, use_collective=True, use_gather=True):
    nsc = nloc // SCN
    nbatch = nsc // BATCH
    eloc = nloc * K
    nc = bacc.Bacc("TRN2", target_bir_lowering=False, debug=False,
                   num_devices=ncores)

    def din(name, shape, dt):
        return nc.dram_tensor(name, shape, dt, kind="ExternalInput").ap()

    eT = din("eT", [128, eloc], F32R)
    hb = din("hb", [nloc * ncores, 128], BF16)
    idx = din("idx", [128, eloc // 16], I16)
    ha1 = din("ha1", [128, nloc], F32R)
    aw1e = din("aw1e", [128, 128], F32R)
    aw1h = din("aw1h", [128, 128], BF16)
    ident = din("identw", [128, 128], F32R)
    identf = din("identf", [128, 128], F32)
    aw2 = din("aw2", [128, 128], F32R)
    aw3r = din("aw3r", [128, 128], F32R)
    nw1e = din("nw1e", [128, 128], F32R)
    nw1h = din("nw1h", [128, 128], BF16)
    nw2 = din("nw2", [128, 128], F32R)
    wp = din("wp", [128, 128], F32R)
    gw1 = din("gw1", [128, 128], F32)
    gw2 = din("gw2", [128, 128], F32)
    gw3 = din("gw3", [128, 128], F32)
    # bias vectors packed [128, 7]: ab2, nb1, nb2, bp, gb1, gb2, gb3/2
    bvec = din("bvec", [128, 7], F32)
    out = nc.dram_tensor("out", [nloc, 128], F32, kind="ExternalOutput").ap()

    with tile.TileContext(nc) as tc, ExitStack() as ctx:
        wpool = ctx.enter_context(tc.tile_pool(name="wpool", bufs=1))
        per = ctx.enter_context(tc.tile_pool(name="per", bufs=1))
        epool = ctx.enter_context(tc.tile_pool(name="epool", bufs=BATCH + 2))
        jpool = ctx.enter_context(tc.tile_pool(name="jpool", bufs=2 * BATCH))
        ppool = ctx.enter_context(tc.tile_pool(name="ppool", bufs=BATCH + 1))
        xpool = ctx.enter_context(tc.tile_pool(name="xpool", bufs=3))
        y2pool = ctx.enter_context(tc.tile_pool(name="y2pool", bufs=3))
        opool = ctx.enter_context(tc.tile_pool(name="opool", bufs=2))
        psG = ctx.enter_context(tc.tile_pool(name="psG", bufs=4, space="PSUM"))
        dram = ctx.enter_context(tc.tile_pool(name="dram", bufs=1, space="DRAM"))

        # --- load weights / persistent data -------------------------------
        def wtile(src, dt):
            t = wpool.tile([128, src.shape[1]], dt, name=f"w_{src.tensor.name}")
            nc.sync.dma_start(t[:], src[:])
            return t

        w_aw1e = wtile(aw1e, F32R)
        w_aw1h = wtile(aw1h, BF16)
        w_ident = wtile(ident, F32R)
        w_identf = wtile(identf, F32)
        w_aw2 = wtile(aw2, F32R)
        w_aw3r = wtile(aw3r, F32R)
        w_nw1e = wtile(nw1e, F32R)
        w_nw1h = wtile(nw1h, BF16)
        w_nw2 = wtile(nw2, F32R)
        w_wp = wtile(wp, F32R)
        w_gw1 = wtile(gw1, F32)
        w_gw2 = wtile(gw2, F32)
        w_gw3 = wtile(gw3, F32)
        w_bias = wtile(bvec, F32)
        b_ab2 = w_bias[:, 0:1]
        b_nb1 = w_bias[:, 1:2]
        b_nb2 = w_bias[:, 2:3]
        b_bp = w_bias[:, 3:4]
        b_gb1 = w_bias[:, 4:5]
        b_gb2 = w_bias[:, 5:6]
        b_gb3h = w_bias[:, 6:7]

        ha1_sb = per.tile([128, nloc], F32R)
        nc.sync.dma_start(ha1_sb[:], ha1[:])
        idx_sb = per.tile([128, eloc // 16], I16)
        nc.sync.dma_start(idx_sb[:], idx[:])
        u_sb = per.tile([128, nloc], F32)
        den_sb = per.tile([128, nloc], F32)
        hh_sb = per.tile([128, nloc], F32)

        nc.gpsimd.load_library(_mlp_lib)

        def stage_psum(name):
            # a stage is a pair of 2-bank tiles (4 blocks of 480 total)
            return [psG.tile([128, 2, 512], F32, name=f"{name}_h{h}", tag="G")
                    for h in range(2)]

        def mm_blk(ps, b, w, rhs, start, stop):
            nc.tensor.matmul(ps[b // 2][:, b % 2, 0:BLK], w[:], rhs,
                             start=start, stop=stop)

        def half(ps, h):
            return ps[h][:, :, 0:BLK]

        def grouped(t, n):
            return t[:].rearrange("p (n k) -> p n k", n=n)

        # --- main loop ----------------------------------------------------
        def fetch_batch(bi):
            e_tiles, j_tiles = [], []
            for sj in range(BATCH):
                sc = bi * BATCH + sj
                c0 = sc * SCC
                et = epool.tile([128, SCC], F32R, name=f"et{bi}_{sj}", tag="et")
                nc.sync.dma_start(et[:], eT[:, c0:c0 + SCC])
                jt = jpool.tile([128, SCC], BF16, name=f"jt{bi}_{sj}", tag="jt")
                if use_gather:
                    # transpose-mode dma_gather is limited to <=768 idxs/call
                    for gj in range(SCC // GC):
                        i0 = sc * (SCC // 16) + gj * (GC // 16)
                        nc.gpsimd.dma_gather(
                            jt[:, gj * GC:(gj + 1) * GC].unsqueeze(1), hb[:],
                            idx_sb[:, i0:i0 + GC // 16],
                            GC, GC, 128, transpose=True)
                else:
                    nc.gpsimd.memset(jt[:], 1.0)
                e_tiles.append(et)
                j_tiles.append(jt)
            return e_tiles, j_tiles

        pending = fetch_batch(0)
        for bi in range(nbatch):
            e_tiles, j_tiles = pending
            p_tiles = []

            # ---- attention phase over the batch --------------------------
            for sj in range(BATCH):
                sc = bi * BATCH + sj
                n0 = sc * SCN
                et, jt = e_tiles[sj], j_tiles[sj]
                pt = ppool.tile([128, SCC], BF16, name=f"pt{bi}_{sj}", tag="pt")
                p_tiles.append(pt)

                a1 = stage_psum(f"a1_{sc}")
                for b in range(NBLK):
                    mm_blk(a1, b, w_aw1e, et[:, b * BLK:(b + 1) * BLK], True, False)
                for b in range(NBLK):
                    mm_blk(a1, b, w_aw1h, jt[:, b * BLK:(b + 1) * BLK], False, False)
                for b in range(NBLK):
                    rep = ha1_sb[:, n0 + b * BLKN:n0 + (b + 1) * BLKN] \
                        .unsqueeze(2).broadcast_to([128, BLKN, K])
                    nc.tensor.matmul(
                        a1[b // 2][:, b % 2, 0:BLK].rearrange(
                            "p (n k) -> p n k", n=BLKN),
                        w_ident[:], rep, start=False, stop=True)

                x1 = xpool.tile([128, SCC], F32R, name=f"x1_{sc}", tag="x")
                for h in range(2):
                    xv = x1[:, h * 2 * BLK:(h + 1) * 2 * BLK] \
                        .rearrange("p (b c) -> p b c", b=2)
                    if h == 1 and sc % 2 == 1:
                        nc.scalar.activation(xv, half(a1, h), AF.Relu)
                    else:
                        nc.vector.tensor_scalar_max(xv, half(a1, h), 0.0)

                a2 = stage_psum(f"a2_{sc}")
                for b in range(NBLK):
                    mm_blk(a2, b, w_aw2, x1[:, b * BLK:(b + 1) * BLK], True, True)
                x2 = xpool.tile([128, SCC], F32R, name=f"x2_{sc}", tag="x")
                for h in range(2):
                    xv = x2[:, h * 2 * BLK:(h + 1) * 2 * BLK] \
                        .rearrange("p (b c) -> p b c", b=2)
                    if h == 0 and sc % 2 == 0:
                        nc.scalar.activation(xv, half(a2, h), AF.Relu, bias=b_ab2)
                    else:
                        nc.vector.tensor_scalar(xv, half(a2, h), b_ab2, 0.0,
                                                OP.add, OP.max)

                a3 = stage_psum(f"a3_{sc}")
                for b in range(NBLK):
                    mm_blk(a3, b, w_aw3r, x2[:, b * BLK:(b + 1) * BLK], True, True)
                for h in range(2):
                    pv = pt[:, h * 2 * BLK:(h + 1) * 2 * BLK] \
                        .rearrange("p (b c) -> p b c", b=2)
                    nc.scalar.activation(pv, half(a3, h), AF.Exp, scale=SCALE)

                nc.vector.reduce_sum(den_sb[:, n0:n0 + SCN], grouped(pt, SCN),
                                     axis=mybir.AxisListType.X)

            # prefetch the next batch while the node phase runs
            if bi + 1 < nbatch:
                pending = fetch_batch(bi + 1)

            # ---- node-value phase over the batch -------------------------
            for sj in range(BATCH):
                sc = bi * BATCH + sj
                n0 = sc * SCN
                et, jt, pt = e_tiles[sj], j_tiles[sj], p_tiles[sj]

                n1 = stage_psum(f"n1_{sc}")
                for b in range(NBLK):
                    mm_blk(n1, b, w_nw1e, et[:, b * BLK:(b + 1) * BLK], True, False)
                for b in range(NBLK):
                    mm_blk(n1, b, w_nw1h, jt[:, b * BLK:(b + 1) * BLK], False, True)
                y1 = xpool.tile([128, SCC], F32R, name=f"y1_{sc}", tag="x")
                for h in range(2):
                    yv = y1[:, h * 2 * BLK:(h + 1) * 2 * BLK] \
                        .rearrange("p (b c) -> p b c", b=2)
                    nc.scalar.activation(yv, half(n1, h), AF.Gelu, bias=b_nb1)

                n2 = stage_psum(f"n2_{sc}")
                for b in range(NBLK):
                    mm_blk(n2, b, w_nw2, y1[:, b * BLK:(b + 1) * BLK], True, True)
                y2 = y2pool.tile([128, SCC], BF16, name=f"y2_{sc}", tag="y2")
                for h in range(2):
                    yv = y2[:, h * 2 * BLK:(h + 1) * 2 * BLK] \
                        .rearrange("p (b c) -> p b c", b=2)
                    nc.scalar.activation(yv, half(n2, h), AF.Gelu, bias=b_nb2)

                # p <- p * y2, then K-group sum into u
                eng = nc.gpsimd if sc % 2 == 0 else nc.vector
                eng.tensor_tensor(pt[:], y2[:], pt[:], OP.mult)
                nc.vector.reduce_sum(u_sb[:, n0:n0 + SCN], grouped(pt, SCN),
                                     axis=mybir.AxisListType.X)

        # --- tail: normalize, project, gate, transpose, store -------------
        deni = per.tile([128, nloc], F32)
        nc.vector.reciprocal(deni[:], den_sb[:])
        ubar = per.tile([128, nloc], F32R)
        nc.vector.tensor_tensor(ubar[:], u_sb[:], deni[:], OP.mult)

        nhb = nloc // 512
        hps = stage_psum("hps")
        for b in range(nhb):
            nc.tensor.matmul(hps[b // 2][:, b % 2, :], w_wp[:],
                             ubar[:, b * 512:(b + 1) * 512],
                             start=True, stop=True)
        for h in range((nhb + 1) // 2):
            w = min(2, nhb - h * 2)
            nc.scalar.activation(
                hh_sb[:, h * 1024:h * 1024 + w * 512]
                .rearrange("p (b c) -> p b c", b=w),
                hps[h][:, 0:w, :], AF.Identity, bias=b_bp)

        csum = per.tile([128, 1], F32)
        nc.vector.reduce_sum(csum[:], hh_sb[:], axis=mybir.AxisListType.X)

        cin = dram.tile([128, 1], F32)
        cout = dram.tile([128, 1], F32,
                         addr_space="Shared" if ncores > 4 else "Local")
        nc.sync.dma_start(cin[:], csum[:])
        if use_collective:
            nc.gpsimd.collective_compute(
                "AllReduce", OP.add,
                replica_groups=[list(range(ncores))],
                ins=[cin[:].opt()], outs=[cout[:].opt()])
        else:
            nc.sync.dma_start(cout[:], cin[:])
        call = per.tile([128, 1], F32)
        nc.sync.dma_start(call[:], cout[:])

        # gate MLP (redundant on every core), sigmoid via tanh
        gps = stage_psum("gps")
        nc.tensor.matmul(gps[0][:, 0, 0:1], w_gw1[:], call[:], start=True, stop=True)
        g1 = per.tile([128, 1], F32)
        nc.scalar.activation(g1[:], gps[0][:, 0, 0:1], AF.Relu, bias=b_gb1,
                             scale=1.0 / float(ncores * nloc))
        nc.tensor.matmul(gps[0][:, 1, 0:1], w_gw2[:], g1[:], start=True, stop=True)
        g2 = per.tile([128, 1], F32)
        nc.scalar.activation(g2[:], gps[0][:, 1, 0:1], AF.Relu, bias=b_gb2)
        nc.tensor.matmul(gps[1][:, 0, 0:1], w_gw3[:], g2[:], start=True, stop=True)
        gth = per.tile([128, 1], F32)
        nc.scalar.activation(gth[:], gps[1][:, 0, 0:1], AF.Tanh, bias=b_gb3h,
                             scale=0.5)
        gv = per.tile([128, 1], F32)
        nc.vector.tensor_scalar(gv[:], gth[:], 0.5, 0.5, OP.mult, OP.add)

        nc.vector.tensor_scalar(hh_sb[:], hh_sb[:], gv[:], None, OP.mult)

        # transpose [128 feat, nloc] -> [nloc, 128] and store
        for q in range(nloc // 512):
            tps = stage_psum(f"tps_{q}")
            for b in range(4):
                nc.tensor.transpose(tps[b // 2][:, b % 2, 0:128],
                                    hh_sb[:, q * 512 + b * 128:q * 512 + (b + 1) * 128],
                                    w_identf[:])
            ot = opool.tile([128, 4, 128], F32, name=f"ot_{q}", tag="ot")
            for h in range(2):
                nc.vector.tensor_copy(ot[:, h * 2:(h + 1) * 2, :],
                                      tps[h][:, :, 0:128])
            # DRAM rows q*512 + b*128 + p, col f  <-  ot[p, b, f]
            dview = out[q * 512:(q + 1) * 512, :] \
                .rearrange("(b p) f -> p b f", b=4)
            nc.sync.dma_start(dview, ot[:])

    nc.compile()
    return nc


def _prep_inputs(h, e, aw1, ab1, aw2, ab2, aw3, ab3,
                 nw1, nb1, nw2, nb2, nw3, nb3, thw,
                 gw1, gb1, gw2, gb2, gw3, gb3,
                 edge_idx, batch_idx, ncores=NCORES, nloc=NLOC):
    n = ncores * nloc
    eloc = nloc * K
    src = np.asarray(edge_idx[0])
    assert np.array_equal(src, np.repeat(np.arange(n, dtype=src.dtype), K)), \
        "kernel assumes edge_idx[0] == repeat(arange(N), K)"
    assert np.all(np.asarray(batch_idx) == 0), "kernel assumes batch_idx == 0"
    dst = np.asarray(edge_idx[1]).astype(np.int16)

    h = np.asarray(h, np.float32)
    e = np.asarray(e, np.float32)
    eT = np.ascontiguousarray(e.T)                      # [128, E]
    hb = np.ascontiguousarray(h.astype(ml_dtypes.bfloat16))
    ha1 = np.ascontiguousarray((h @ np.asarray(aw1)[:D] + np.asarray(ab1)).T)
    wp = np.asarray(nw3, np.float32) @ np.asarray(thw, np.float32)
    bp = np.asarray(nb3, np.float32) @ np.asarray(thw, np.float32)
    aw3r = np.ascontiguousarray(np.tile(np.asarray(aw3, np.float32), (1, 128)))
    identw = np.eye(128, dtype=np.float32)

    bvec = np.stack([
        np.asarray(ab2, np.float32), np.asarray(nb1, np.float32),
        np.asarray(nb2, np.float32), bp,
        np.asarray(gb1, np.float32), np.asarray(gb2, np.float32),
        np.asarray(gb3, np.float32) * 0.5,
    ], axis=1)                                          # [128, 7]

    common = {
        "hb": hb,
        "aw1e": np.ascontiguousarray(np.asarray(aw1, np.float32)[D:2 * D]),
        "aw1h": np.ascontiguousarray(
            np.asarray(aw1, np.float32)[2 * D:3 * D].astype(ml_dtypes.bfloat16)),
        "identw": identw, "identf": identw,
        "aw2": np.asarray(aw2, np.float32),
        "aw3r": aw3r,
        "nw1e": np.ascontiguousarray(np.asarray(nw1, np.float32)[:D]),
        "nw1h": np.ascontiguousarray(
            np.asarray(nw1, np.float32)[D:2 * D].astype(ml_dtypes.bfloat16)),
        "nw2": np.asarray(nw2, np.float32),
        "wp": wp,
        "gw1": np.asarray(gw1, np.float32),
        "gw2": np.asarray(gw2, np.float32),
        "gw3": np.asarray(gw3, np.float32),
        "bvec": np.ascontiguousarray(bvec),
    }

    in_maps = []
    for c in range(ncores):
        dc = dst[c * eloc:(c + 1) * eloc]
        iw = np.ascontiguousarray(np.tile(dc.reshape(-1, 16).T, (8, 1)))
        m = dict(common)
        m["eT"] = np.ascontiguousarray(eT[:, c * eloc:(c + 1) * eloc])
        m["idx"] = iw
        m["ha1"] = np.ascontiguousarray(ha1[:, c * nloc:(c + 1) * nloc])
        in_maps.append(m)
    return in_maps


def kernel(**inputs):
    key = "full"
    if key not in _CACHE:
        _CACHE[key] = _build()
    nc = _CACHE[key]
    in_maps = _prep_inputs(**inputs)
    res = run_bass_kernel_spmd(nc, in_maps, core_ids=list(range(NCORES)))
    return np.concatenate([res.results[c]["out"] for c in range(NCORES)], axis=0)


# revision 19
# speedup vs baseline: 10307.0056x; 1.0048x over previous
"""PiGNNLayer Trainium2 Bass kernel.

Computes the reference nn_PiGNNLayer graph-attention layer on 8 NeuronCores.

Sharding: core c owns nodes [c*N/8, (c+1)*N/8) and their contiguous K=30-edge
blocks.  All MLPs / softmax / weighted sums are local to a node's edge block;
the scatter-mean for the gate needs one 512-byte AllReduce.

On-chip layout is feature-major: activations live as [128 features, edges]
tiles so every Linear layer is a weights-stationary matmul
(out_T = W.T @ x_T via matmul(out, lhsT=W, rhs=x_T)).  Edge features e are
pre-transposed on the host; h_dst rows are gathered on-device with
dma_gather(transpose=True) (bf16, 16-bit-granular transpose) straight into
feature-major tiles.  h_src is the node's own row repeated K times, so its
first-layer contribution (h @ aw1[:128] + ab1) is precomputed per node on the
host and added into the layer-1 PSUM with an identity matmul whose rhs uses a
stride-0 column-repeat access pattern.

Each MLP stage accumulates into a 4-bank PSUM tile [128, 4, 512] (one 480-col
node-aligned matmul per bank) and is evacuated to SBUF by a single wide
ACT/DVE instruction reading the strided [128, 4, 480] view, which amortizes
the ~352-cycle per-instruction engine overhead.

Softmax over the K=30 neighbors skips the max-subtraction (logits are O(1) by
construction), exponentiates attention logits that were replicated across all
128 partitions by tiling the aw3 column 128x, and defers normalization to
after the attention-weighted K-sum.  node_mlp layer 3 and to_h are fused into
a single node-level matmul with W' = nw3 @ thw (host-precomputed) because the
einsum commutes with the last linear layer (and sum(att)=1 handles the bias).
"""

import sys
import os

for _p in ("/opt/trn_rl_repo",):
    if _p not in sys.path and os.path.isdir(_p):
        sys.path.insert(0, _p)

import numpy as np
import ml_dtypes
from contextlib import ExitStack

import concourse.bass as bass
import concourse.bacc as bacc
import concourse.tile as tile
import concourse.mybir as mybir
from concourse.bass_utils import run_bass_kernel_spmd
from concourse.library_config import mlp as _mlp_lib

AF = mybir.ActivationFunctionType
OP = mybir.AluOpType
F32 = mybir.dt.float32
F32R = mybir.dt.float32r
BF16 = mybir.dt.bfloat16
I16 = mybir.dt.int16

# Problem shape (hardcoded per spec).
N, K, D, H = 16384, 30, 128, 1
NCORES = 8
NLOC = N // NCORES            # nodes per core
ELOC = NLOC * K               # edges per core
BLKN = 16                     # nodes per matmul block
BLK = BLKN * K                # 480 free-dim columns per matmul block
SCN = 64                      # nodes per super-chunk
SCC = SCN * K                 # 1920 columns per super-chunk
NBLK = SCC // BLK             # 4 blocks per super-chunk
NSC = NLOC // SCN             # super-chunks per core
BATCH = 4                     # super-chunks per att/node phase batch
GC = 640                      # idxs per dma_gather call (HW limit <= 768)
SCALE = 1.0 / float(np.sqrt(D // H))

_CACHE = {}


def _build(ncores=NCORES, nloc=NLOC, use_collective=True, use_gather=True):
    nsc = nloc // SCN
    nbatch = nsc // BATCH
    eloc = nloc * K
    nc = bacc.Bacc("TRN2", target_bir_lowering=False, debug=False,
                   num_devices=ncores)

    def din(name, shape, dt):
        return nc.dram_tensor(name, shape, dt, kind="ExternalInput").ap()

    eT = din("eT", [128, eloc], F32R)
    hb = din("hb", [nloc * ncores, 128], BF16)
    idx = din("idx", [128, eloc // 16], I16)
    ha1 = din("ha1", [128, nloc], F32R)
    aw1e = din("aw1e", [128, 128], F32R)
    aw1h = din("aw1h", [128, 128], BF16)
    ident = din("identw", [128, 128], F32R)
    identf = din("identf", [128, 128], F32)
    aw2 = din("aw2", [128, 128], F32R)
    aw3r = din("aw3r", [128, 128], F32R)
    nw1e = din("nw1e", [128, 128], F32R)
    nw1h = din("nw1h", [128, 128], BF16)
    nw2 = din("nw2", [128, 128], F32R)
    wp = din("wp", [128, 128], F32R)
    gw1 = din("gw1", [128, 128], F32)
    gw2 = din("gw2", [128, 128], F32)
    gw3 = din("gw3", [128, 128], F32)
    # bias vectors packed [128, 7]: ab2, nb1, nb2, bp, gb1, gb2, gb3/2
    bvec = din("bvec", [128, 7], F32)
    out = nc.dram_tensor("out", [nloc, 128], F32, kind="ExternalOutput").ap()

    with tile.TileContext(nc) as tc, ExitStack() as ctx:
        wpool = ctx.enter_context(tc.tile_pool(name="wpool", bufs=1))
        per = ctx.enter_context(tc.tile_pool(name="per", bufs=1))
        epool = ctx.enter_context(tc.tile_pool(name="epool", bufs=BATCH + 2))
        jpool = ctx.enter_context(tc.tile_pool(name="jpool", bufs=2 * BATCH))
        ppool = ctx.enter_context(tc.tile_pool(name="ppool", bufs=BATCH + 1))
        xpool = ctx.enter_context(tc.tile_pool(name="xpool", bufs=3))
        y2pool = ctx.enter_context(tc.tile_pool(name="y2pool", bufs=3))
        opool = ctx.enter_context(tc.tile_pool(name="opool", bufs=2))
        psG = ctx.enter_context(tc.tile_pool(name="psG", bufs=4, space="PSUM"))
        dram = ctx.enter_context(tc.tile_pool(name="dram", bufs=1, space="DRAM"))

        # --- load weights / persistent data -------------------------------
        def wtile(src, dt):
            t = wpool.tile([128, src.shape[1]], dt, name=f"w_{src.tensor.name}")
            nc.sync.dma_start(t[:], src[:])
            return t

        w_aw1e = wtile(aw1e, F32R)
        w_aw1h = wtile(aw1h, BF16)
        w_ident = wtile(ident, F32R)
        w_identf = wtile(identf, F32)
        w_aw2 = wtile(aw2, F32R)
        w_aw3r = wtile(aw3r, F32R)
        w_nw1e = wtile(nw1e, F32R)
        w_nw1h = wtile(nw1h, BF16)
        w_nw2 = wtile(nw2, F32R)
        w_wp = wtile(wp, F32R)
        w_gw1 = wtile(gw1, F32)
        w_gw2 = wtile(gw2, F32)
        w_gw3 = wtile(gw3, F32)
        w_bias = wtile(bvec, F32)
        b_ab2 = w_bias[:, 0:1]
        b_nb1 = w_bias[:, 1:2]
        b_nb2 = w_bias[:, 2:3]
        b_bp = w_bias[:, 3:4]
        b_gb1 = w_bias[:, 4:5]
        b_gb2 = w_bias[:, 5:6]
        b_gb3h = w_bias[:, 6:7]

        ha1_sb = per.tile([128, nloc], F32R)
        nc.sync.dma_start(ha1_sb[:], ha1[:])
        idx_sb = per.tile([128, eloc // 16], I16)
        nc.sync.dma_start(idx_sb[:], idx[:])
        u_sb = per.tile([128, nloc], F32)
        den_sb = per.tile([128, nloc], F32)
        hh_sb = per.tile([128, nloc], F32)

        nc.gpsimd.load_library(_mlp_lib)

        def stage_psum(name):
            # a stage is a pair of 2-bank tiles (4 blocks of 480 total)
            return [psG.tile([128, 2, 512], F32, name=f"{name}_h{h}", tag="G")
                    for h in range(2)]

        def mm_blk(ps, b, w, rhs, start, stop):
            nc.tensor.matmul(ps[b // 2][:, b % 2, 0:BLK], w[:], rhs,
                             start=start, stop=stop)

        def half(ps, h):
            return ps[h][:, :, 0:BLK]

        def grouped(t, n):
            return t[:].rearrange("p (n k) -> p n k", n=n)

        # --- main loop ----------------------------------------------------
        def fetch_batch(bi):
            e_tiles, j_tiles = [], []
            for sj in range(BATCH):
                sc = bi * BATCH + sj
                c0 = sc * SCC
                et = epool.tile([128, SCC], F32R, name=f"et{bi}_{sj}", tag="et")
                nc.sync.dma_start(et[:], eT[:, c0:c0 + SCC])
                jt = jpool.tile([128, SCC], BF16, name=f"jt{bi}_{sj}", tag="jt")
                if use_gather:
                    # transpose-mode dma_gather is limited to <=768 idxs/call
                    for gj in range(SCC // GC):
                        i0 = sc * (SCC // 16) + gj * (GC // 16)
                        nc.gpsimd.dma_gather(
                            jt[:, gj * GC:(gj + 1) * GC].unsqueeze(1), hb[:],
                            idx_sb[:, i0:i0 + GC // 16],
                            GC, GC, 128, transpose=True)
                else:
                    nc.gpsimd.memset(jt[:], 1.0)
                e_tiles.append(et)
                j_tiles.append(jt)
            return e_tiles, j_tiles

        pending = fetch_batch(0)
        for bi in range(nbatch):
            e_tiles, j_tiles = pending
            p_tiles = []

            # ---- attention phase over the batch --------------------------
            for sj in range(BATCH):
                sc = bi * BATCH + sj
                n0 = sc * SCN
                et, jt = e_tiles[sj], j_tiles[sj]
                pt = ppool.tile([128, SCC], BF16, name=f"pt{bi}_{sj}", tag="pt")
                p_tiles.append(pt)

                a1 = stage_psum(f"a1_{sc}")
                for b in range(NBLK):
                    rep = ha1_sb[:, n0 + b * BLKN:n0 + (b + 1) * BLKN] \
                        .unsqueeze(2).broadcast_to([128, BLKN, K])
                    nc.tensor.matmul(
                        a1[b // 2][:, b % 2, 0:BLK].rearrange(
                            "p (n k) -> p n k", n=BLKN),
                        w_ident[:], rep, start=True, stop=False)
                for b in range(NBLK):
                    mm_blk(a1, b, w_aw1e, et[:, b * BLK:(b + 1) * BLK], False, False)
                for b in range(NBLK):
                    mm_blk(a1, b, w_aw1h, jt[:, b * BLK:(b + 1) * BLK], False, True)

                x1 = xpool.tile([128, SCC], F32R, name=f"x1_{sc}", tag="x")
                for h in range(2):
                    xv = x1[:, h * 2 * BLK:(h + 1) * 2 * BLK] \
                        .rearrange("p (b c) -> p b c", b=2)
                    if h == 1 and sc % 2 == 1:
                        nc.scalar.activation(xv, half(a1, h), AF.Relu)
                    else:
                        nc.vector.tensor_scalar_max(xv, half(a1, h), 0.0)

                a2 = stage_psum(f"a2_{sc}")
                for b in range(NBLK):
                    mm_blk(a2, b, w_aw2, x1[:, b * BLK:(b + 1) * BLK], True, True)
                x2 = xpool.tile([128, SCC], F32R, name=f"x2_{sc}", tag="x")
                for h in range(2):
                    xv = x2[:, h * 2 * BLK:(h + 1) * 2 * BLK] \
                        .rearrange("p (b c) -> p b c", b=2)
                    if h == 0 and sc % 2 == 0:
                        nc.scalar.activation(xv, half(a2, h), AF.Relu, bias=b_ab2)
                    else:
                        nc.vector.tensor_scalar(xv, half(a2, h), b_ab2, 0.0,
                                                OP.add, OP.max)

                a3 = stage_psum(f"a3_{sc}")
                for b in range(NBLK):
                    mm_blk(a3, b, w_aw3r, x2[:, b * BLK:(b + 1) * BLK], True, True)
                for h in range(2):
                    pv = pt[:, h * 2 * BLK:(h + 1) * 2 * BLK] \
                        .rearrange("p (b c) -> p b c", b=2)
                    nc.scalar.activation(pv, half(a3, h), AF.Exp, scale=SCALE)

                nc.vector.reduce_sum(den_sb[:, n0:n0 + SCN], grouped(pt, SCN),
                                     axis=mybir.AxisListType.X)

            # prefetch the next batch while the node phase runs
            if bi + 1 < nbatch:
                pending = fetch_batch(bi + 1)

            # ---- node-value phase over the batch -------------------------
            for sj in range(BATCH):
                sc = bi * BATCH + sj
                n0 = sc * SCN
                et, jt, pt = e_tiles[sj], j_tiles[sj], p_tiles[sj]

                n1 = stage_psum(f"n1_{sc}")
                for b in range(NBLK):
                    mm_blk(n1, b, w_nw1e, et[:, b * BLK:(b + 1) * BLK], True, False)
                for b in range(NBLK):
                    mm_blk(n1, b, w_nw1h, jt[:, b * BLK:(b + 1) * BLK], False, True)
                y1 = xpool.tile([128, SCC], F32R, name=f"y1_{sc}", tag="x")
                for h in range(2):
                    yv = y1[:, h * 2 * BLK:(h + 1) * 2 * BLK] \
                        .rearrange("p (b c) -> p b c", b=2)
                    nc.scalar.activation(yv, half(n1, h), AF.Gelu, bias=b_nb1)

                n2 = stage_psum(f"n2_{sc}")
                for b in range(NBLK):
                    mm_blk(n2, b, w_nw2, y1[:, b * BLK:(b + 1) * BLK], True, True)
                y2 = y2pool.tile([128, SCC], BF16, name=f"y2_{sc}", tag="y2")
                for h in range(2):
                    yv = y2[:, h * 2 * BLK:(h + 1) * 2 * BLK] \
                        .rearrange("p (b c) -> p b c", b=2)
                    nc.scalar.activation(yv, half(n2, h), AF.Gelu, bias=b_nb2)

                # p <- p * y2, then K-group sum into u
                eng = nc.gpsimd if sc % 2 == 0 else nc.vector
                eng.tensor_tensor(pt[:], y2[:], pt[:], OP.mult)
                nc.vector.reduce_sum(u_sb[:, n0:n0 + SCN], grouped(pt, SCN),
                                     axis=mybir.AxisListType.X)

        # --- tail: normalize, project, gate, transpose, store -------------
        deni = per.tile([128, nloc], F32)
        nc.vector.reciprocal(deni[:], den_sb[:])
        ubar = per.tile([128, nloc], F32R)
        nc.vector.tensor_tensor(ubar[:], u_sb[:], deni[:], OP.mult)

        nhb = nloc // 512
        hps = stage_psum("hps")
        for b in range(nhb):
            nc.tensor.matmul(hps[b // 2][:, b % 2, :], w_wp[:],
                             ubar[:, b * 512:(b + 1) * 512],
                             start=True, stop=True)
        for h in range((nhb + 1) // 2):
            w = min(2, nhb - h * 2)
            nc.scalar.activation(
                hh_sb[:, h * 1024:h * 1024 + w * 512]
                .rearrange("p (b c) -> p b c", b=w),
                hps[h][:, 0:w, :], AF.Identity, bias=b_bp)

        csum = per.tile([128, 1], F32)
        nc.vector.reduce_sum(csum[:], hh_sb[:], axis=mybir.AxisListType.X)

        cin = dram.tile([128, 1], F32)
        cout = dram.tile([128, 1], F32,
                         addr_space="Shared" if ncores > 4 else "Local")
        nc.sync.dma_start(cin[:], csum[:])
        if use_collective:
            nc.gpsimd.collective_compute(
                "AllReduce", OP.add,
                replica_groups=[list(range(ncores))],
                ins=[cin[:].opt()], outs=[cout[:].opt()])
        else:
            nc.sync.dma_start(cout[:], cin[:])
        call = per.tile([128, 1], F32)
        nc.sync.dma_start(call[:], cout[:])

        # gate MLP (redundant on every core), sigmoid via tanh
        gps = stage_psum("gps")
        nc.tensor.matmul(gps[0][:, 0, 0:1], w_gw1[:], call[:], start=True, stop=True)
        g1 = per.tile([128, 1], F32)
        nc.scalar.activation(g1[:], gps[0][:, 0, 0:1], AF.Relu, bias=b_gb1,
                             scale=1.0 / float(ncores * nloc))
        nc.tensor.matmul(gps[0][:, 1, 0:1], w_gw2[:], g1[:], start=True, stop=True)
        g2 = per.tile([128, 1], F32)
        nc.scalar.activation(g2[:], gps[0][:, 1, 0:1], AF.Relu, bias=b_gb2)
        nc.tensor.matmul(gps[1][:, 0, 0:1], w_gw3[:], g2[:], start=True, stop=True)
        gth = per.tile([128, 1], F32)
        nc.scalar.activation(gth[:], gps[1][:, 0, 0:1], AF.Tanh, bias=b_gb3h,
                             scale=0.5)
        gv = per.tile([128, 1], F32)
        nc.vector.tensor_scalar(gv[:], gth[:], 0.5, 0.5, OP.mult, OP.add)

        nc.vector.tensor_scalar(hh_sb[:], hh_sb[:], gv[:], None, OP.mult)

        # transpose [128 feat, nloc] -> [nloc, 128] and store
        for q in range(nloc // 512):
            tps = stage_psum(f"tps_{q}")
            for b in range(4):
                nc.tensor.transpose(tps[b // 2][:, b % 2, 0:128],
                                    hh_sb[:, q * 512 + b * 128:q * 512 + (b + 1) * 128],
                                    w_identf[:])
            ot = opool.tile([128, 4, 128], F32, name=f"ot_{q}", tag="ot")
            for h in range(2):
                nc.vector.tensor_copy(ot[:, h * 2:(h + 1) * 2, :],
                                      tps[h][:, :, 0:128])
            # DRAM rows q*512 + b*128 + p, col f  <-  ot[p, b, f]
            dview = out[q * 512:(q + 1) * 512, :] \
                .rearrange("(b p) f -> p b f", b=4)
            nc.sync.dma_start(dview, ot[:])

    nc.compile()
    return nc


def _prep_inputs(h, e, aw1, ab1, aw2, ab2, aw3, ab3,
                 nw1, nb1, nw2, nb2, nw3, nb3, thw,
                 gw1, gb1, gw2, gb2, gw3, gb3,
                 edge_idx, batch_idx, ncores=NCORES, nloc=NLOC):
    n = ncores * nloc
    eloc = nloc * K
    src = np.asarray(edge_idx[0])
    assert np.array_equal(src, np.repeat(np.arange(n, dtype=src.dtype), K)), \
        "kernel assumes edge_idx[0] == repeat(arange(N), K)"
    assert np.all(np.asarray(batch_idx) == 0), "kernel assumes batch_idx == 0"
    dst = np.asarray(edge_idx[1]).astype(np.int16)

    h = np.asarray(h, np.float32)
    e = np.asarray(e, np.float32)
    eT = np.ascontiguousarray(e.T)                      # [128, E]
    hb = np.ascontiguousarray(h.astype(ml_dtypes.bfloat16))
    ha1 = np.ascontiguousarray((h @ np.asarray(aw1)[:D] + np.asarray(ab1)).T)
    wp = np.asarray(nw3, np.float32) @ np.asarray(thw, np.float32)
    bp = np.asarray(nb3, np.float32) @ np.asarray(thw, np.float32)
    aw3r = np.ascontiguousarray(np.tile(np.asarray(aw3, np.float32), (1, 128)))
    identw = np.eye(128, dtype=np.float32)

    bvec = np.stack([
        np.asarray(ab2, np.float32), np.asarray(nb1, np.float32),
        np.asarray(nb2, np.float32), bp,
        np.asarray(gb1, np.float32), np.asarray(gb2, np.float32),
        np.asarray(gb3, np.float32) * 0.5,
    ], axis=1)                                          # [128, 7]

    common = {
        "hb": hb,
        "aw1e": np.ascontiguousarray(np.asarray(aw1, np.float32)[D:2 * D]),
        "aw1h": np.ascontiguousarray(
            np.asarray(aw1, np.float32)[2 * D:3 * D].astype(ml_dtypes.bfloat16)),
        "identw": identw, "identf": identw,
        "aw2": np.asarray(aw2, np.float32),
        "aw3r": aw3r,
        "nw1e": np.ascontiguousarray(np.asarray(nw1, np.float32)[:D]),
        "nw1h": np.ascontiguousarray(
            np.asarray(nw1, np.float32)[D:2 * D].astype(ml_dtypes.bfloat16)),
        "nw2": np.asarray(nw2, np.float32),
        "wp": wp,
        "gw1": np.asarray(gw1, np.float32),
        "gw2": np.asarray(gw2, np.float32),
        "gw3": np.asarray(gw3, np.float32),
        "bvec": np.ascontiguousarray(bvec),
    }

    in_maps = []
    for c in range(ncores):
        dc = dst[c * eloc:(c + 1) * eloc]
        iw = np.ascontiguousarray(np.tile(dc.reshape(-1, 16).T, (8, 1)))
        m = dict(common)
        m["eT"] = np.ascontiguousarray(eT[:, c * eloc:(c + 1) * eloc])
        m["idx"] = iw
        m["ha1"] = np.ascontiguousarray(ha1[:, c * nloc:(c + 1) * nloc])
        in_maps.append(m)
    return in_maps


def kernel(**inputs):
    key = "full"
    if key not in _CACHE:
        _CACHE[key] = _build()
    nc = _CACHE[key]
    in_maps = _prep_inputs(**inputs)
    res = run_bass_kernel_spmd(nc, in_maps, core_ids=list(range(NCORES)))
    return np.concatenate([res.results[c]["out"] for c in range(NCORES)], axis=0)
